# revision 2
# baseline (speedup 1.0000x reference)
"""MDTA (Restormer channel attention) Bass/Tile kernel for 8 Trainium2 cores.

Sharding: spatial. Core c handles batch b=c//4, image rows 64*(c%4) .. +64.
The channel attention Gram G = Q K^T and the L2 norms are sums over spatial
positions, so each core accumulates per-head joint [q|k] 96x96 Gram partials
locally and one tiny (2 x 96 x 768 fp32) AllReduce combines them; the
normalization (F.normalize) is applied afterwards as row/col scaling of G.

v2 changes vs v1:
  - v channels unpadded (384, proj order). COUT 1280 -> 1152 (9 o-tiles).
  - attention A folded into project_out on device: P^T = blockdiag(A)^T-free
    per-head matmuls (lhsT = softmax L slice, rhs = per-head projT), so
    phase 3 is a single GEMM y = P^T.T @ v_dw. No A^T transposes, no
    separate apply pass.
  - depthwise 3x3 split 5 DVE taps / 4 PE diag-matmul taps; scalar engine
    evacuates the PE partial, DVE does the final 2x-mode merge add.
  - gram DMA transposes issued from the (idle) sync engine; q/k 1x1-conv
    weights prescaled (undone in the dw taps).

Device channel layout (host pre-permutes all weights to match):
  o-tiles 0..5  (768 ch): per head h: [q_h (48) | k_h (48)] interleaved -> the
                joint per-head Gram block is contiguous, and its diagonal
                gives ssq/ssk for the L2 norms.
  o-tiles 6..8  (384 ch): v in natural (proj input) order.
"""

import sys

for p in ("/opt/trn_rl_repo", "/opt/pypackages"):
    if p not in sys.path:
        sys.path.insert(0, p)

import numpy as np
import ml_dtypes

import concourse.bass as bass
import concourse.mybir as mybir
import concourse.tile as tile
import concourse.bacc as bacc
from concourse.bass_utils import run_bass_kernel_spmd

BF16 = ml_dtypes.bfloat16

B, DIM, HGT, WID = 2, 384, 256, 256
HEADS = 8
HD = DIM // HEADS  # 48
N_CORES = 8
ROWS = HGT // 4  # 64 output rows per core
WP = WID + 2  # 258 padded width
HP = ROWS + 2  # 66 padded rows per core
NBLK = 8  # row super-blocks per core
BR = ROWS // NBLK  # 8 output rows per block
BP = BR + 2  # 10 padded rows per block
BN = BR * WID  # 2048 output cols per block
NLOC = ROWS * WID  # 16384 output cols per core

CQK = 2 * DIM  # 768 interleaved q/k channels
CV = DIM  # 384 v channels (unpadded, proj order)
COUT = CQK + CV  # 1152 total device channels
OT_QK = CQK // 128  # 6
OT_V = CV // 128  # 3
OT = OT_QK + OT_V  # 9
CT = DIM // 128  # 3 x c-tiles
GSTRIDE = WP * BP  # 2580 cols per block GEMM
# GEMM moving chunks: 16B-aligned starts (DoubleRow AP requirement)
GCHUNKS = ((0, 864), (864, 864), (1728, 852))
PE_TAPS = (5, 6, 7, 8)  # depthwise taps on PE (diag matmuls)
DVE_TAPS = (0, 1, 2, 3, 4)  # depthwise taps on DVE
X8W = 17040  # HP*WP (17028) padded to %16 for the fp8 interleave stride
X8C = 2592  # per-block fp8 chunk width (2580 padded to %16)

F32 = mybir.dt.float32
BF = mybir.dt.bfloat16
F8 = mybir.dt.float8e4
FP8 = ml_dtypes.float8_e4m3
QKSCALE = 128.0  # qk weight prescale into fp8's good range (undone in dw)


def _build_program():
    nc = bacc.Bacc(
        "TRN2",
        target_bir_lowering=False,
        debug=False,
        num_devices=N_CORES,
    )

    xp = nc.dram_tensor("xp", [CT, 128, HP * WP], BF, kind="ExternalInput")
    wqkvT = nc.dram_tensor("wqkvT", [CT, 128, COUT], BF, kind="ExternalInput")
    wdw = nc.dram_tensor("wdw", [OT, 128, 9], F32, kind="ExternalInput")
    wprojH = nc.dram_tensor("wprojH", [HD, HEADS * DIM], BF, kind="ExternalInput")
    tempb = nc.dram_tensor("tempb", [HD, HEADS], F32, kind="ExternalInput")
    eyeb = nc.dram_tensor("eyeb", [128, 128], BF, kind="ExternalInput")
    eyem = nc.dram_tensor("eyem", [96, 768], F32, kind="ExternalInput")
    mask8 = nc.dram_tensor("mask8", [HEADS, DIM], BF, kind="ExternalInput")
    wdiag = nc.dram_tensor(
        "wdiag", [OT, 128, len(PE_TAPS) * 128], BF, kind="ExternalInput"
    )
    y = nc.dram_tensor("y", [CT, 128, NLOC], F32, kind="ExternalOutput")

    AOP = mybir.AluOpType
    ACT = mybir.ActivationFunctionType

    with tile.TileContext(nc) as tc:
        with (
            tc.tile_pool(name="const", bufs=1) as constp,
            tc.tile_pool(name="xin", bufs=1) as xinp,
            tc.tile_pool(name="pre", bufs=1) as prep,
            tc.tile_pool(name="acc", bufs=1) as accp,
            tc.tile_pool(name="qkc", bufs=1) as qkcp,
            tc.tile_pool(name="vc", bufs=1) as vcp,
            tc.tile_pool(name="qkt", bufs=2) as qktp,
            tc.tile_pool(name="small", bufs=1) as smallp,
            tc.tile_pool(name="vin", bufs=2) as vinp,
            tc.tile_pool(name="yout", bufs=2) as youtp,
            tc.tile_pool(name="psA", bufs=3, space="PSUM") as psA,
            tc.tile_pool(name="psC", bufs=2, space="PSUM") as psC,
            tc.tile_pool(name="psG", bufs=1, space="PSUM") as psG,
            tc.tile_pool(name="dram", bufs=1, space="DRAM") as dramp,
        ):
            # ---- resident constants --------------------------------------
            wq_sb = []
            for ct in range(CT):
                t = constp.tile([128, COUT], BF, tag=f"wq{ct}")
                nc.sync.dma_start(t[:], wqkvT[ct])
                wq_sb.append(t)
            wdw_sb = []
            for ot in range(OT):
                t = constp.tile([128, 9], F32, tag=f"wdw{ot}")
                nc.sync.dma_start(t[:], wdw[ot])
                wdw_sb.append(t)
            wprojH_sb = constp.tile([HD, HEADS * DIM], BF, tag="wprojH")
            nc.sync.dma_start(wprojH_sb[:], wprojH[:])
            tempb_sb = constp.tile([HD, HEADS], F32, tag="tempb")
            nc.sync.dma_start(tempb_sb[:], tempb[:])
            eyeb_sb = constp.tile([128, 128], BF, tag="eyeb")
            nc.sync.dma_start(eyeb_sb[:], eyeb[:])
            eyem_sb = constp.tile([96, 768], F32, tag="eyem")
            nc.sync.dma_start(eyem_sb[:], eyem[:])
            ones_sb = constp.tile([HEADS, HD], BF, tag="ones")
            nc.vector.memset(ones_sb[:], 1.0)
            mask8_sb = constp.tile([HEADS, DIM], BF, tag="mask8")
            nc.sync.dma_start(mask8_sb[:], mask8[:])
            wdiag_sb = []
            for ot in range(OT):
                t = constp.tile([128, len(PE_TAPS) * 128], BF, tag=f"wdiag{ot}")
                nc.sync.dma_start(t[:], wdiag[ot])
                wdiag_sb.append(t)

            v_dram = dramp.tile([OT_V, 128, NLOC], BF)
            qk_dram = dramp.tile([OT_QK, 128, NLOC], BF)
            cc_in = dramp.tile([96, 768], F32)
            cc_out = dramp.tile([96, 768], F32)

            # Gram accumulators: 2 banks x [96, 4*96] (4 heads per bank)
            gram_ps = [
                psG.tile([96, 384], F32, tag=f"g{i}", name=f"gram{i}")
                for i in range(2)
            ]

            # ---- phase 1: stream row blocks ------------------------------
            qkd2 = qk_dram[:, :, :].rearrange("t p n -> (t p) n")

            def _gram_block(kb):
                for half in range(BN // 128):
                    nt = kb * (BN // 128) + half
                    qkT = qktp.tile([128, CQK], BF, tag="qkT")
                    nc.sync.dma_start_transpose(
                        qkT[:], qkd2[:, nt * 128 : (nt + 1) * 128]
                    )
                    first = kb == 0 and half == 0
                    last = kb == NBLK - 1 and half == BN // 128 - 1
                    for h in range(HEADS):
                        nc.tensor.matmul(
                            gram_ps[h // 4][:, (h % 4) * 96 : (h % 4) * 96 + 96],
                            lhsT=qkT[:, h * 96 : h * 96 + 96],
                            rhs=qkT[:, h * 96 : h * 96 + 96],
                            start=first,
                            stop=last,
                            skip_group_check=True,
                        )

            for k in range(NBLK):
                # x rows 8k .. 8k+10 (padded indexing), all 3 c-tiles
                x_sb = []
                for ct in range(CT):
                    t = xinp.tile([128, GSTRIDE], BF, tag=f"x{ct}")
                    nc.sync.dma_start(
                        t[:], xp[ct][:, k * BR * WP : k * BR * WP + GSTRIDE]
                    )
                    x_sb.append(t)
                # qkv pointwise GEMM for the block (qk weights are
                # prescaled by QKSCALE; undone in the dw taps).
                pre_sb = []
                for ot in range(OT):
                    t = prep.tile([128, GSTRIDE], BF, tag=f"pre{ot}")
                    pre_sb.append(t)
                GCH = 430
                for ot in range(OT):
                    for g in range(GSTRIDE // GCH):
                        ps = psA.tile([128, 512], F32, tag="gemm")
                        for ct in range(CT):
                            nc.tensor.matmul(
                                ps[:, :GCH],
                                lhsT=wq_sb[ct][:, ot * 128 : (ot + 1) * 128],
                                rhs=x_sb[ct][:, g * GCH : (g + 1) * GCH],
                                start=(ct == 0),
                                stop=(ct == CT - 1),
                            )
                        nc.scalar.copy(
                            pre_sb[ot][:, g * GCH : (g + 1) * GCH], ps[:, :GCH]
                        )

                # depthwise 3x3: DVE_TAPS on DVE (tensor_scalar +
                # tensor_tensor), PE_TAPS as diag-matmul PSUM accumulation
                # with shifted moving-operand APs; scalar evacuates the PE
                # partial, gpsimd merges the two halves.
                for ot in range(OT):
                    pre_r = pre_sb[ot][:].rearrange("p (r w) -> p r w", w=WP)
                    if ot < OT_QK:
                        dst = qkcp.tile([128, BN], BF, tag=f"qk{ot}")
                    else:
                        dst = vcp.tile([128, BN], BF, tag=f"v{ot - OT_QK}")
                    accA = accp.tile([128, BN], BF, tag="accA")
                    accB = accp.tile([128, BN], BF, tag="accB", bufs=2)
                    tmp = accp.tile([128, BN], BF, tag="tmp")
                    pp = accp.tile([128, BN], BF, tag="pp", bufs=2)
                    pair = [accA, accB]
                    for i, s in enumerate(DVE_TAPS):
                        dh, dw = s // 3, s % 3
                        srcap = pre_r[:, dh : dh + BR, dw : dw + WID]
                        wcol = wdw_sb[ot][:, s : s + 1]
                        cur, nxt = pair[(i + 1) % 2], pair[i % 2]
                        if i == 0:
                            nc.vector.tensor_scalar_mul(nxt[:], srcap, wcol)
                        else:
                            nc.vector.tensor_scalar_mul(tmp[:], srcap, wcol)
                            nc.vector.tensor_tensor(
                                nxt[:], cur[:], tmp[:], AOP.add
                            )
                    accD = pair[(len(DVE_TAPS) + 1) % 2]
                    for g in range(4):
                        pc = psC.tile([128, 512], F32, tag="conv")
                        for i, s in enumerate(PE_TAPS):
                            dh, dw = s // 3, s % 3
                            rhs = pre_r[
                                :, dh + 2 * g : dh + 2 * g + 2, dw : dw + WID
                            ]
                            nc.tensor.matmul(
                                pc[:],
                                lhsT=wdiag_sb[ot][:, i * 128 : (i + 1) * 128],
                                rhs=rhs,
                                start=(i == 0),
                                stop=(i == len(PE_TAPS) - 1),
                            )
                        nc.scalar.copy(pp[:, g * 512 : (g + 1) * 512], pc[:])
                    nc.vector.tensor_tensor(dst[:], accD[:], pp[:], AOP.add)
                    if ot >= OT_QK:
                        vt = ot - OT_QK
                        nc.sync.dma_start(
                            v_dram[vt][:, k * BN : (k + 1) * BN], dst[:]
                        )
                    else:
                        nc.sync.dma_start(
                            qk_dram[ot][:, k * BN : (k + 1) * BN], dst[:]
                        )

                # xbar-transpose q/k n-tiles from DRAM, accumulate Grams.
                # Lagged one block (process block k-1 here, block NBLK-1
                # after the loop) so the gram matmuls never stall PE on the
                # merge->DMA->transpose chain of the current block.
                if k > 0:
                    _gram_block(k - 1)

            _gram_block(NBLK - 1)

            # ---- phase 1.5: per-batch-group AllReduce --------------------
            # prefetch phase-3's first v chunk behind the collective
            v_sb0 = []
            for t in range(OT_V):
                vt_ = vinp.tile([128, 2048], BF, tag=f"vin{t}")
                nc.scalar.dma_start(vt_[:], v_dram[t][:, 0:2048])
                v_sb0.append(vt_)
            ccin_sb = smallp.tile([96, 768], F32, tag="ccin")
            for g in range(2):
                nc.scalar.copy(
                    ccin_sb[:, g * 384 : (g + 1) * 384], gram_ps[g][:]
                )
            nc.sync.dma_start(cc_in[:], ccin_sb[:])
            nc.gpsimd.collective_compute(
                "AllReduce",
                AOP.add,
                replica_groups=[[0, 1, 2, 3], [4, 5, 6, 7]],
                ins=[cc_in.opt()],
                outs=[cc_out.opt()],
            )
            gred = smallp.tile([96, 768], F32, tag="gred")
            nc.sync.dma_start(gred[:], cc_out[:])

            # ---- phase 2: norms, scaling, softmax ------------------------
            # diag -> per-channel sum of squares [96(joint c), 8(head)]
            dm = smallp.tile([96, 768], F32, tag="dm")
            nc.vector.tensor_tensor(dm[:], gred[:], eyem_sb[:], AOP.mult)
            dsum = smallp.tile([96, HEADS], F32, tag="dsum")
            nc.vector.tensor_reduce(
                dsum[:],
                dm[:].rearrange("p (h d) -> p h d", d=96),
                axis=mybir.AxisListType.X,
                op=AOP.add,
            )
            norms = smallp.tile([96, HEADS], F32, tag="norms")
            nc.scalar.sqrt(norms[:], dsum[:])
            nc.vector.tensor_scalar_max(norms[:], norms[:], 1e-12)
            rsc = smallp.tile([96, HEADS], F32, tag="rsc")
            nc.vector.reciprocal(rsc[:], norms[:])

            # rk broadcast [48, h*48+d] = rsc[48+d, h]:
            # transpose rsc -> rscT [8, 96]; rkrep[h', (h,d)] = rscT[h', 48+d]
            # masked by delta(h'=h); then ones[8,48].T @ rkrep sums out h'.
            rscb = smallp.tile([96, HEADS], BF, tag="rscb")
            nc.vector.tensor_copy(rscb[:], rsc[:])
            rscT_ps = psC.tile([128, 128], BF, tag="conv")
            nc.tensor.transpose(
                rscT_ps[:HEADS, :96], rscb[:], eyeb_sb[:96, :96]
            )
            rscT = smallp.tile([HEADS, 96], BF, tag="rscT")
            nc.vector.tensor_copy(rscT[:], rscT_ps[:HEADS, :96])
            rkrep = smallp.tile([HEADS, DIM], BF, tag="rkrep")
            mask3d = mask8_sb[:].rearrange("p (h d) -> p h d", d=HD)
            rk3d = rscT[:, HD : 2 * HD].rearrange("p (o d) -> p o d", o=1)
            mask3d, rk3d = bass.broadcast_tensor_aps(mask3d, rk3d)
            nc.vector.tensor_tensor(
                rkrep[:].rearrange("p (h d) -> p h d", d=HD),
                mask3d,
                rk3d,
                AOP.mult,
            )
            rkb_ps = psA.tile([128, 512], F32, tag="gemm")
            nc.tensor.matmul(
                rkb_ps[:HD, :DIM],
                lhsT=ones_sb[:],
                rhs=rkrep[:],
                start=True,
                stop=True,
            )
            # logits L[c, h, d] = G_qk * rk * (temp_h * rq)
            L = smallp.tile([HD, DIM], F32, tag="L")
            gqk = gred[0:HD].rearrange("p (h d) -> p h d", d=96)[:, :, HD : 2 * HD]
            nc.vector.tensor_tensor(
                L[:].rearrange("p (h d) -> p h d", d=HD),
                gqk,
                rkb_ps[:HD, :DIM].rearrange("p (h d) -> p h d", d=HD),
                AOP.mult,
            )
            tsc = smallp.tile([HD, HEADS], F32, tag="tsc")
            nc.vector.tensor_tensor(tsc[:], tempb_sb[:], rsc[0:HD, :], AOP.mult)
            for h in range(HEADS):
                nc.vector.tensor_scalar_mul(
                    L[:, h * HD : (h + 1) * HD],
                    L[:, h * HD : (h + 1) * HD],
                    tsc[:, h : h + 1],
                )
            # softmax over d (free dim, per 48-block)
            mx = smallp.tile([HD, HEADS], F32, tag="mx")
            nc.vector.tensor_reduce(
                mx[:],
                L[:].rearrange("p (h d) -> p h d", d=HD),
                axis=mybir.AxisListType.X,
                op=AOP.max,
            )
            for h in range(HEADS):
                nc.vector.tensor_scalar_sub(
                    L[:, h * HD : (h + 1) * HD],
                    L[:, h * HD : (h + 1) * HD],
                    mx[:, h : h + 1],
                )
            nc.scalar.activation(L[:], L[:], ACT.Exp)
            sm = smallp.tile([HD, HEADS], F32, tag="sm")
            nc.vector.tensor_reduce(
                sm[:],
                L[:].rearrange("p (h d) -> p h d", d=HD),
                axis=mybir.AxisListType.X,
                op=AOP.add,
            )
            rs = smallp.tile([HD, HEADS], F32, tag="rs")
            nc.vector.reciprocal(rs[:], sm[:])
            for h in range(HEADS):
                nc.vector.tensor_scalar_mul(
                    L[:, h * HD : (h + 1) * HD],
                    L[:, h * HD : (h + 1) * HD],
                    rs[:, h : h + 1],
                )
            Lb = smallp.tile([HD, DIM], BF, tag="Lb")
            nc.vector.tensor_copy(Lb[:], L[:])

            # ---- phase 2.5: fold A into proj: P^T[48h+j, o] =
            # sum_i A_h[i,j] * proj[o, 48h+i]; lhsT = Lb[:, h*48+j] slice,
            # rhs = wprojH[:, h*384 : (h+1)*384]. Each head computed at
            # partition 0 in PSUM, then scalar-copied into the P^T tiles
            # (split where a head straddles a 128-partition boundary).
            PT_sb = []
            for ct in range(CT):
                t = smallp.tile([128, DIM], BF, tag=f"PT{ct}")
                PT_sb.append(t)
            for h in range(HEADS):
                ps_h = psA.tile([128, 512], F32, tag="gemm", name=f"ptps{h}")
                nc.tensor.matmul(
                    ps_h[:HD, :DIM],
                    lhsT=Lb[:, h * HD : (h + 1) * HD],
                    rhs=wprojH_sb[:, h * DIM : (h + 1) * DIM],
                    start=True,
                    stop=True,
                )
                stage = smallp.tile([HD, DIM], BF, tag="ptstage", bufs=2)
                nc.scalar.copy(stage[:], ps_h[:HD, :DIM])
                r0 = h * HD
                ct0 = r0 // 128
                split = (ct0 + 1) * 128
                if r0 + HD <= split:
                    pieces = [(ct0, r0 - ct0 * 128, 0, HD)]
                else:
                    pieces = [
                        (ct0, r0 - ct0 * 128, 0, split - r0),
                        (ct0 + 1, 0, split - r0, r0 + HD - split),
                    ]
                # engines need 32-aligned partition bases; DMA does not.
                for ct, row0, joff, jlen in pieces:
                    nc.sync.dma_start(
                        PT_sb[ct][row0 : row0 + jlen, :],
                        stage[joff : joff + jlen, :],
                    )

            # ---- phase 3: fused (proj @ A) @ v_dw GEMM -------------------
            VCH = 2048  # v reload chunk

            for nt in range(NLOC // 512):
                if nt % (VCH // 512) == 0:
                    if nt == 0:
                        v_sb = v_sb0
                    else:
                        v_sb = []
                        for t in range(OT_V):
                            vt_ = vinp.tile([128, VCH], BF, tag=f"vin{t}")
                            nc.sync.dma_start(
                                vt_[:],
                                v_dram[t][:, nt * 512 : nt * 512 + VCH],
                            )
                            v_sb.append(vt_)
                off = (nt % (VCH // 512)) * 512
                for po in range(CT):
                    ps = psA.tile([128, 512], F32, tag="gemm", name=f"y{nt}_{po}")
                    for t in range(OT_V):
                        nc.tensor.matmul(
                            ps[:, :512],
                            lhsT=PT_sb[t][:, po * 128 : (po + 1) * 128],
                            rhs=v_sb[t][:, off : off + 512],
                            start=(t == 0),
                            stop=(t == OT_V - 1),
                        )
                    ysb = youtp.tile([128, 512], F32, tag="ysb", name=f"ys{nt}_{po}", bufs=4)
                    if po % 2 == 0:
                        nc.scalar.copy(ysb[:], ps[:, :512])
                        nc.scalar.dma_start(
                            y[po][:, nt * 512 : (nt + 1) * 512], ysb[:]
                        )
                    else:
                        nc.vector.tensor_copy(ysb[:], ps[:, :512])
                        nc.gpsimd.dma_start(
                            y[po][:, nt * 512 : (nt + 1) * 512], ysb[:]
                        )

    nc.compile()
    return nc


_NC = None


def _get_program():
    global _NC
    if _NC is None:
        _NC = _build_program()
    return _NC


def _prep_weights(qkv_w, dw_w, proj_w, log_temp):
    """Host-side weight permutation/padding. Returns dict of shared inputs."""
    qkv_w = np.asarray(qkv_w, np.float32)
    dw_w = np.asarray(dw_w, np.float32).reshape(3 * DIM, 9)
    proj_w = np.asarray(proj_w, np.float32)
    temp = np.log1p(np.exp(np.asarray(log_temp, np.float32).reshape(HEADS)))
    temp = temp + 1e-6

    # permutation: first 768 = per head [q_h | k_h]; then v in natural order
    perm = np.concatenate(
        [
            np.concatenate([np.arange(h * HD, (h + 1) * HD),
                            DIM + np.arange(h * HD, (h + 1) * HD)])
            for h in range(HEADS)
        ]
        + [2 * DIM + np.arange(DIM)]
    )
    wq = qkv_w[perm].copy()
    wd = dw_w[perm].copy()
    # prescale qk 1x1 weights into fp8's range; undo in the dw taps
    wq[:CQK] *= QKSCALE
    wd[:CQK] /= QKSCALE

    wqkvT = np.ascontiguousarray(wq.T.reshape(CT, 128, COUT)).astype(BF16)
    wdw = np.ascontiguousarray(wd.reshape(OT, 128, 9))


    # wprojH[i, h*384 + o] = proj_w[o, 48h + i]
    wprojH = np.zeros((HD, HEADS * DIM), np.float32)
    for h in range(HEADS):
        wprojH[:, h * DIM : (h + 1) * DIM] = proj_w[:, h * HD : (h + 1) * HD].T
    wprojH = wprojH.astype(BF16)

    tempb = np.broadcast_to(temp[None, :], (HD, HEADS)).copy()
    eyeb = np.eye(128, dtype=np.float32).astype(BF16)
    eyem = np.tile(np.eye(96, dtype=np.float32), (1, 8)).copy()
    mask8 = np.repeat(np.eye(HEADS, dtype=np.float32), HD, axis=1).astype(BF16)
    wdiag = np.zeros((OT, len(PE_TAPS), 128, 128), np.float32)
    for ot in range(OT):
        for i, s in enumerate(PE_TAPS):
            np.fill_diagonal(wdiag[ot, i], wd[ot * 128 : (ot + 1) * 128, s])
    # sbuf layout: [128 part(k), ntaps*128 free(s, m)]
    wdiag = np.ascontiguousarray(wdiag.transpose(0, 2, 1, 3)).reshape(
        OT, 128, len(PE_TAPS) * 128
    ).astype(BF16)
    return {
        "wqkvT": wqkvT,
        "wdw": wdw,
        "wprojH": wprojH,
        "tempb": tempb,
        "eyeb": eyeb,
        "eyem": eyem,
        "mask8": mask8,
        "wdiag": wdiag,
    }


def _prep_x(x):
    """Per-core padded x chunks: bf16 [CT, 128, HP*WP] and the fp8
    interleaved copy of c-tiles 0,1 ([128, 2*X8W])."""
    x = np.asarray(x, np.float32)
    chunks = []
    for c in range(N_CORES):
        b, r0 = c // 4, ROWS * (c % 4)
        buf = np.zeros((DIM, HP, WP), np.float32)
        lo, hi = max(r0 - 1, 0), min(r0 + ROWS + 1, HGT)
        buf[:, lo - (r0 - 1) : hi - (r0 - 1), 1 : WID + 1] = x[b, :, lo:hi, :]
        flat = buf.reshape(CT, 128, HP * WP)
        chunks.append(np.ascontiguousarray(flat).astype(BF16))
    return chunks


def _run(x, qkv_w, dw_w, proj_w, log_temp, trace=False):
    nc = _get_program()
    shared = _prep_weights(qkv_w, dw_w, proj_w, log_temp)
    xchunks = _prep_x(x)
    in_maps = [
        {**shared, "xp": xchunks[c]} for c in range(N_CORES)
    ]
    res = run_bass_kernel_spmd(
        nc, in_maps, core_ids=list(range(N_CORES)), trace=trace
    )
    out = np.empty((B, DIM, HGT, WID), np.float32)
    for c in range(N_CORES):
        b, r0 = c // 4, ROWS * (c % 4)
        yc = res.results[c]["y"].reshape(DIM, ROWS, WID)
        out[b, :, r0 : r0 + ROWS, :] = yc
    return out, res


def kernel(x, qkv_w, dw_w, proj_w, log_temp):
    out, _ = _run(x, qkv_w, dw_w, proj_w, log_temp, trace=False)
    return out


# revision 3
# speedup vs baseline: 1.0062x; 1.0062x over previous
"""MDTA (Restormer channel attention) Bass/Tile kernel for 8 Trainium2 cores.

Sharding: spatial. Core c handles batch b=c//4, image rows 64*(c%4) .. +64.
The channel attention Gram G = Q K^T and the L2 norms are sums over spatial
positions, so each core accumulates per-head joint [q|k] 96x96 Gram partials
locally and one tiny (2 x 96 x 768 fp32) AllReduce combines them; the
normalization (F.normalize) is applied afterwards as row/col scaling of G.

v2 changes vs v1:
  - v channels unpadded (384, proj order). COUT 1280 -> 1152 (9 o-tiles).
  - attention A folded into project_out on device: P^T = blockdiag(A)^T-free
    per-head matmuls (lhsT = softmax L slice, rhs = per-head projT), so
    phase 3 is a single GEMM y = P^T.T @ v_dw. No A^T transposes, no
    separate apply pass.
  - depthwise 3x3 split 5 DVE taps / 4 PE diag-matmul taps; scalar engine
    evacuates the PE partial, DVE does the final 2x-mode merge add.
  - q/k 1x1-conv GEMM in fp8 (DoubleRow over c-tiles 0+1, weights
    prescaled by QKSCALE, undone in the dw taps; full-width matmuls only
    -- column-split DoubleRow corrupts PSUM).
  - per-ot GEMM->depthwise interleave on PE so the DVE tap chain + merge
    never stalls on a whole-block GEMM section.
  - gram DMA transposes issued from the (idle) sync engine.

Device channel layout (host pre-permutes all weights to match):
  o-tiles 0..5  (768 ch): per head h: [q_h (48) | k_h (48)] interleaved -> the
                joint per-head Gram block is contiguous, and its diagonal
                gives ssq/ssk for the L2 norms.
  o-tiles 6..8  (384 ch): v in natural (proj input) order.
"""

import sys

for p in ("/opt/trn_rl_repo", "/opt/pypackages"):
    if p not in sys.path:
        sys.path.insert(0, p)

import numpy as np
import ml_dtypes

import concourse.bass as bass
import concourse.mybir as mybir
import concourse.tile as tile
import concourse.bacc as bacc
from concourse.bass_utils import run_bass_kernel_spmd

BF16 = ml_dtypes.bfloat16

B, DIM, HGT, WID = 2, 384, 256, 256
HEADS = 8
HD = DIM // HEADS  # 48
N_CORES = 8
ROWS = HGT // 4  # 64 output rows per core
WP = WID + 2  # 258 padded width
HP = ROWS + 2  # 66 padded rows per core
NBLK = 8  # row super-blocks per core
BR = ROWS // NBLK  # 8 output rows per block
BP = BR + 2  # 10 padded rows per block
BN = BR * WID  # 2048 output cols per block
NLOC = ROWS * WID  # 16384 output cols per core

CQK = 2 * DIM  # 768 interleaved q/k channels
CV = DIM  # 384 v channels (unpadded, proj order)
COUT = CQK + CV  # 1152 total device channels
OT_QK = CQK // 128  # 6
OT_V = CV // 128  # 3
OT = OT_QK + OT_V  # 9
CT = DIM // 128  # 3 x c-tiles
GSTRIDE = WP * BP  # 2580 cols per block GEMM
# GEMM moving chunks: 16B-aligned starts (DoubleRow AP requirement)
GCHUNKS = ((0, 864), (864, 864), (1728, 852))
PE_TAPS = (5, 6, 7, 8)  # depthwise taps on PE (diag matmuls)
DVE_TAPS = (0, 1, 2, 3, 4)  # depthwise taps on DVE
X8W = 17040  # HP*WP (17028) padded to %16 for the fp8 interleave stride
X8C = 2592  # per-block fp8 chunk width (2580 padded to %16)

F32 = mybir.dt.float32
BF = mybir.dt.bfloat16
F8 = mybir.dt.float8e4
FP8 = ml_dtypes.float8_e4m3
QKSCALE = 128.0  # qk weight prescale into fp8's good range (undone in dw)


def _build_program():
    nc = bacc.Bacc(
        "TRN2",
        target_bir_lowering=False,
        debug=False,
        num_devices=N_CORES,
    )

    xp = nc.dram_tensor("xp", [CT, 128, HP * WP], BF, kind="ExternalInput")
    xp8 = nc.dram_tensor("xp8", [128, 2 * X8W], F8, kind="ExternalInput")
    wq8 = nc.dram_tensor("wq8", [128, 2 * CQK], F8, kind="ExternalInput")
    wqkvT = nc.dram_tensor("wqkvT", [CT, 128, COUT], BF, kind="ExternalInput")
    wdw = nc.dram_tensor("wdw", [OT, 128, 9], F32, kind="ExternalInput")
    wprojH = nc.dram_tensor("wprojH", [HD, HEADS * DIM], BF, kind="ExternalInput")
    tempb = nc.dram_tensor("tempb", [HD, HEADS], F32, kind="ExternalInput")
    eyeb = nc.dram_tensor("eyeb", [128, 128], BF, kind="ExternalInput")
    eyem = nc.dram_tensor("eyem", [96, 768], F32, kind="ExternalInput")
    mask8 = nc.dram_tensor("mask8", [HEADS, DIM], BF, kind="ExternalInput")
    wdiag = nc.dram_tensor(
        "wdiag", [OT, 128, len(PE_TAPS) * 128], BF, kind="ExternalInput"
    )
    y = nc.dram_tensor("y", [CT, 128, NLOC], F32, kind="ExternalOutput")

    AOP = mybir.AluOpType
    ACT = mybir.ActivationFunctionType

    with tile.TileContext(nc) as tc:
        with (
            tc.tile_pool(name="const", bufs=1) as constp,
            tc.tile_pool(name="xin", bufs=1) as xinp,
            tc.tile_pool(name="pre", bufs=1) as prep,
            tc.tile_pool(name="acc", bufs=1) as accp,
            tc.tile_pool(name="qkc", bufs=1) as qkcp,
            tc.tile_pool(name="vc", bufs=1) as vcp,
            tc.tile_pool(name="qkt", bufs=2) as qktp,
            tc.tile_pool(name="small", bufs=1) as smallp,
            tc.tile_pool(name="vin", bufs=2) as vinp,
            tc.tile_pool(name="yout", bufs=2) as youtp,
            tc.tile_pool(name="psA", bufs=3, space="PSUM") as psA,
            tc.tile_pool(name="psC", bufs=2, space="PSUM") as psC,
            tc.tile_pool(name="psG", bufs=1, space="PSUM") as psG,
            tc.tile_pool(name="dram", bufs=1, space="DRAM") as dramp,
        ):
            # ---- resident constants --------------------------------------
            wq_sb = []
            for ct in range(CT):
                t = constp.tile([128, COUT], BF, tag=f"wq{ct}")
                nc.sync.dma_start(t[:], wqkvT[ct])
                wq_sb.append(t)
            wq8_sb = constp.tile([128, 2 * CQK], F8, tag="wq8")
            nc.sync.dma_start(wq8_sb[:], wq8[:])
            wdw_sb = []
            for ot in range(OT):
                t = constp.tile([128, 9], F32, tag=f"wdw{ot}")
                nc.sync.dma_start(t[:], wdw[ot])
                wdw_sb.append(t)
            wprojH_sb = constp.tile([HD, HEADS * DIM], BF, tag="wprojH")
            nc.sync.dma_start(wprojH_sb[:], wprojH[:])
            tempb_sb = constp.tile([HD, HEADS], F32, tag="tempb")
            nc.sync.dma_start(tempb_sb[:], tempb[:])
            eyeb_sb = constp.tile([128, 128], BF, tag="eyeb")
            nc.sync.dma_start(eyeb_sb[:], eyeb[:])
            eyem_sb = constp.tile([96, 768], F32, tag="eyem")
            nc.sync.dma_start(eyem_sb[:], eyem[:])
            ones_sb = constp.tile([HEADS, HD], BF, tag="ones")
            nc.vector.memset(ones_sb[:], 1.0)
            mask8_sb = constp.tile([HEADS, DIM], BF, tag="mask8")
            nc.sync.dma_start(mask8_sb[:], mask8[:])
            wdiag_sb = []
            for ot in range(OT):
                t = constp.tile([128, len(PE_TAPS) * 128], BF, tag=f"wdiag{ot}")
                nc.sync.dma_start(t[:], wdiag[ot])
                wdiag_sb.append(t)

            v_dram = dramp.tile([OT_V, 128, NLOC], BF)
            qk_dram = dramp.tile([OT_QK, 128, NLOC], BF)
            cc_in = dramp.tile([96, 768], F32)
            cc_out = dramp.tile([96, 768], F32)

            # Gram accumulators: 2 banks x [96, 4*96] (4 heads per bank)
            gram_ps = [
                psG.tile([96, 384], F32, tag=f"g{i}", name=f"gram{i}")
                for i in range(2)
            ]

            # ---- phase 1: stream row blocks ------------------------------
            qkd2 = qk_dram[:, :, :].rearrange("t p n -> (t p) n")

            def _gram_block(kb):
                for half in range(BN // 128):
                    nt = kb * (BN // 128) + half
                    qkT = qktp.tile([128, CQK], BF, tag="qkT")
                    nc.sync.dma_start_transpose(
                        qkT[:], qkd2[:, nt * 128 : (nt + 1) * 128]
                    )
                    first = kb == 0 and half == 0
                    last = kb == NBLK - 1 and half == BN // 128 - 1
                    for h in range(HEADS):
                        nc.tensor.matmul(
                            gram_ps[h // 4][:, (h % 4) * 96 : (h % 4) * 96 + 96],
                            lhsT=qkT[:, h * 96 : h * 96 + 96],
                            rhs=qkT[:, h * 96 : h * 96 + 96],
                            start=first,
                            stop=last,
                            skip_group_check=True,
                        )

            for k in range(NBLK):
                # x rows 8k .. 8k+10 (padded indexing), all 3 c-tiles
                x_sb = []
                for ct in range(CT):
                    t = xinp.tile([128, GSTRIDE], BF, tag=f"x{ct}")
                    nc.sync.dma_start(
                        t[:], xp[ct][:, k * BR * WP : k * BR * WP + GSTRIDE]
                    )
                    x_sb.append(t)
                # fp8 interleaved view of c-tiles 0,1 for the DoubleRow GEMM
                x8_sb = xinp.tile([128, 2 * X8C], F8, tag="x8")
                for j in range(2):
                    nc.sync.dma_start(
                        x8_sb[:, j * X8C : j * X8C + GSTRIDE],
                        xp8[:, j * X8W + k * BR * WP : j * X8W + k * BR * WP + GSTRIDE],
                    )
                x8r = x8_sb[:].rearrange("p (j n) -> p j n", j=2)
                wq8r = wq8_sb[:].rearrange("p (j m) -> p j m", j=2)

                # qkv pointwise GEMM. qk o-tiles: one fp8 DoubleRow matmul
                # (c-tiles 0+1, weights prescaled by QKSCALE; undone in the
                # dw taps) + one bf16 matmul (c-tile 2). v o-tiles: bf16.
                # Chunk starts 16B-aligned (DoubleRow moving-AP rule); a
                # DoubleRow matmul must be full-width (no column splits).
                pre_sb = []
                for ot in range(OT):
                    t = prep.tile([128, GSTRIDE], BF, tag=f"pre{ot}")
                    pre_sb.append(t)
                # per-ot pipeline: GEMM(ot) then depthwise(ot) on PE, so
                # the DVE chain + merge for ot never waits on a whole-block
                # PE GEMM section (kills the phase-1 PE/DVE alternation).
                for ot in range(OT):
                    for g0, glen in ((0, 432), (432, 432), (864, 432),
                                     (1296, 432), (1728, 432), (2160, 420)):
                        ps = psA.tile([128, 512], F32, tag="gemm")
                        if ot < OT_QK:
                            nc.tensor.matmul(
                                ps[:, :glen],
                                lhsT=wq8r[:, :, ot * 128 : (ot + 1) * 128],
                                rhs=x8r[:, :, g0 : g0 + glen],
                                start=True,
                                stop=False,
                                perf_mode=mybir.MatmulPerfMode.DoubleRow,
                            )
                            nc.tensor.matmul(
                                ps[:, :glen],
                                lhsT=wq_sb[2][:, ot * 128 : (ot + 1) * 128],
                                rhs=x_sb[2][:, g0 : g0 + glen],
                                start=False,
                                stop=True,
                            )
                        else:
                            for ct in range(CT):
                                nc.tensor.matmul(
                                    ps[:, :glen],
                                    lhsT=wq_sb[ct][:, ot * 128 : (ot + 1) * 128],
                                    rhs=x_sb[ct][:, g0 : g0 + glen],
                                    start=(ct == 0),
                                    stop=(ct == CT - 1),
                                )
                        nc.scalar.copy(
                            pre_sb[ot][:, g0 : g0 + glen], ps[:, :glen]
                        )

                    # depthwise 3x3 for this ot: DVE_TAPS on DVE, PE_TAPS as
                    # diag-matmul PSUM accumulation; scalar evacuates the PE
                    # partial, DVE merges.
                    pre_r = pre_sb[ot][:].rearrange("p (r w) -> p r w", w=WP)
                    if ot < OT_QK:
                        dst = qkcp.tile([128, BN], BF, tag=f"qk{ot}")
                    else:
                        dst = vcp.tile([128, BN], BF, tag=f"v{ot - OT_QK}")
                    accA = accp.tile([128, BN], BF, tag="accA")
                    accB = accp.tile([128, BN], BF, tag="accB")
                    tmp = accp.tile([128, BN], BF, tag="tmp")
                    pp = accp.tile([128, BN], BF, tag="pp", bufs=2)
                    pair = [accA, accB]
                    for i, s in enumerate(DVE_TAPS):
                        dh, dw = s // 3, s % 3
                        srcap = pre_r[:, dh : dh + BR, dw : dw + WID]
                        wcol = wdw_sb[ot][:, s : s + 1]
                        cur, nxt = pair[(i + 1) % 2], pair[i % 2]
                        if i == 0:
                            nc.vector.tensor_scalar_mul(nxt[:], srcap, wcol)
                        else:
                            nc.vector.tensor_scalar_mul(tmp[:], srcap, wcol)
                            nc.vector.tensor_tensor(
                                nxt[:], cur[:], tmp[:], AOP.add
                            )
                    accD = pair[(len(DVE_TAPS) + 1) % 2]
                    for g in range(4):
                        pc = psC.tile([128, 512], F32, tag="conv")
                        for i, s in enumerate(PE_TAPS):
                            dh, dw = s // 3, s % 3
                            rhs = pre_r[
                                :, dh + 2 * g : dh + 2 * g + 2, dw : dw + WID
                            ]
                            nc.tensor.matmul(
                                pc[:],
                                lhsT=wdiag_sb[ot][:, i * 128 : (i + 1) * 128],
                                rhs=rhs,
                                start=(i == 0),
                                stop=(i == len(PE_TAPS) - 1),
                            )
                        nc.scalar.copy(pp[:, g * 512 : (g + 1) * 512], pc[:])
                    nc.vector.tensor_tensor(dst[:], accD[:], pp[:], AOP.add)
                    if ot >= OT_QK:
                        vt = ot - OT_QK
                        nc.sync.dma_start(
                            v_dram[vt][:, k * BN : (k + 1) * BN], dst[:]
                        )
                    else:
                        nc.sync.dma_start(
                            qk_dram[ot][:, k * BN : (k + 1) * BN], dst[:]
                        )

                # xbar-transpose q/k n-tiles from DRAM, accumulate Grams.
                # Lagged one block (process block k-1 here, block NBLK-1
                # after the loop) so the gram matmuls never stall PE on the
                # merge->DMA->transpose chain of the current block.
                if k > 0:
                    _gram_block(k - 1)

            _gram_block(NBLK - 1)

            # ---- phase 1.5: per-batch-group AllReduce --------------------
            # prefetch phase-3's first two v chunks behind the collective
            v_pre = []
            for g in range(2):
                grp = []
                for t in range(OT_V):
                    vt_ = vinp.tile([128, 2048], BF, tag=f"vin{t}",
                                    name=f"vpre{g}_{t}")
                    nc.scalar.dma_start(
                        vt_[:], v_dram[t][:, g * 2048 : (g + 1) * 2048]
                    )
                    grp.append(vt_)
                v_pre.append(grp)
            ccin_sb = smallp.tile([96, 768], F32, tag="ccin")
            for g in range(2):
                nc.scalar.copy(
                    ccin_sb[:, g * 384 : (g + 1) * 384], gram_ps[g][:]
                )
            nc.sync.dma_start(cc_in[:], ccin_sb[:])
            nc.gpsimd.collective_compute(
                "AllReduce",
                AOP.add,
                replica_groups=[[0, 1, 2, 3], [4, 5, 6, 7]],
                ins=[cc_in.opt()],
                outs=[cc_out.opt()],
            )
            gred = smallp.tile([96, 768], F32, tag="gred")
            nc.sync.dma_start(gred[:], cc_out[:])

            # ---- phase 2: norms, scaling, softmax ------------------------
            # diag -> per-channel sum of squares [96(joint c), 8(head)]
            dm = smallp.tile([96, 768], F32, tag="dm")
            nc.vector.tensor_tensor(dm[:], gred[:], eyem_sb[:], AOP.mult)
            dsum = smallp.tile([96, HEADS], F32, tag="dsum")
            nc.vector.tensor_reduce(
                dsum[:],
                dm[:].rearrange("p (h d) -> p h d", d=96),
                axis=mybir.AxisListType.X,
                op=AOP.add,
            )
            norms = smallp.tile([96, HEADS], F32, tag="norms")
            nc.scalar.sqrt(norms[:], dsum[:])
            nc.vector.tensor_scalar_max(norms[:], norms[:], 1e-12)
            rsc = smallp.tile([96, HEADS], F32, tag="rsc")
            nc.vector.reciprocal(rsc[:], norms[:])

            # rk broadcast [48, h*48+d] = rsc[48+d, h]:
            # transpose rsc -> rscT [8, 96]; rkrep[h', (h,d)] = rscT[h', 48+d]
            # masked by delta(h'=h); then ones[8,48].T @ rkrep sums out h'.
            rscb = smallp.tile([96, HEADS], BF, tag="rscb")
            nc.vector.tensor_copy(rscb[:], rsc[:])
            rscT_ps = psC.tile([128, 128], BF, tag="conv")
            nc.tensor.transpose(
                rscT_ps[:HEADS, :96], rscb[:], eyeb_sb[:96, :96]
            )
            rscT = smallp.tile([HEADS, 96], BF, tag="rscT")
            nc.vector.tensor_copy(rscT[:], rscT_ps[:HEADS, :96])
            rkrep = smallp.tile([HEADS, DIM], BF, tag="rkrep")
            mask3d = mask8_sb[:].rearrange("p (h d) -> p h d", d=HD)
            rk3d = rscT[:, HD : 2 * HD].rearrange("p (o d) -> p o d", o=1)
            mask3d, rk3d = bass.broadcast_tensor_aps(mask3d, rk3d)
            nc.vector.tensor_tensor(
                rkrep[:].rearrange("p (h d) -> p h d", d=HD),
                mask3d,
                rk3d,
                AOP.mult,
            )
            rkb_ps = psA.tile([128, 512], F32, tag="gemm")
            nc.tensor.matmul(
                rkb_ps[:HD, :DIM],
                lhsT=ones_sb[:],
                rhs=rkrep[:],
                start=True,
                stop=True,
            )
            # logits L[c, h, d] = G_qk * rk * (temp_h * rq)
            L = smallp.tile([HD, DIM], F32, tag="L")
            gqk = gred[0:HD].rearrange("p (h d) -> p h d", d=96)[:, :, HD : 2 * HD]
            nc.vector.tensor_tensor(
                L[:].rearrange("p (h d) -> p h d", d=HD),
                gqk,
                rkb_ps[:HD, :DIM].rearrange("p (h d) -> p h d", d=HD),
                AOP.mult,
            )
            tsc = smallp.tile([HD, HEADS], F32, tag="tsc")
            nc.vector.tensor_tensor(tsc[:], tempb_sb[:], rsc[0:HD, :], AOP.mult)
            for h in range(HEADS):
                nc.vector.tensor_scalar_mul(
                    L[:, h * HD : (h + 1) * HD],
                    L[:, h * HD : (h + 1) * HD],
                    tsc[:, h : h + 1],
                )
            # softmax over d (free dim, per 48-block)
            mx = smallp.tile([HD, HEADS], F32, tag="mx")
            nc.vector.tensor_reduce(
                mx[:],
                L[:].rearrange("p (h d) -> p h d", d=HD),
                axis=mybir.AxisListType.X,
                op=AOP.max,
            )
            for h in range(HEADS):
                nc.vector.tensor_scalar_sub(
                    L[:, h * HD : (h + 1) * HD],
                    L[:, h * HD : (h + 1) * HD],
                    mx[:, h : h + 1],
                )
            nc.scalar.activation(L[:], L[:], ACT.Exp)
            sm = smallp.tile([HD, HEADS], F32, tag="sm")
            nc.vector.tensor_reduce(
                sm[:],
                L[:].rearrange("p (h d) -> p h d", d=HD),
                axis=mybir.AxisListType.X,
                op=AOP.add,
            )
            rs = smallp.tile([HD, HEADS], F32, tag="rs")
            nc.vector.reciprocal(rs[:], sm[:])
            for h in range(HEADS):
                nc.vector.tensor_scalar_mul(
                    L[:, h * HD : (h + 1) * HD],
                    L[:, h * HD : (h + 1) * HD],
                    rs[:, h : h + 1],
                )
            Lb = smallp.tile([HD, DIM], BF, tag="Lb")
            nc.vector.tensor_copy(Lb[:], L[:])

            # ---- phase 2.5: fold A into proj: P^T[48h+j, o] =
            # sum_i A_h[i,j] * proj[o, 48h+i]; lhsT = Lb[:, h*48+j] slice,
            # rhs = wprojH[:, h*384 : (h+1)*384]. Each head computed at
            # partition 0 in PSUM, then scalar-copied into the P^T tiles
            # (split where a head straddles a 128-partition boundary).
            PT_sb = []
            for ct in range(CT):
                t = smallp.tile([128, DIM], BF, tag=f"PT{ct}")
                PT_sb.append(t)
            for h in range(HEADS):
                ps_h = psA.tile([128, 512], F32, tag="gemm", name=f"ptps{h}")
                nc.tensor.matmul(
                    ps_h[:HD, :DIM],
                    lhsT=Lb[:, h * HD : (h + 1) * HD],
                    rhs=wprojH_sb[:, h * DIM : (h + 1) * DIM],
                    start=True,
                    stop=True,
                )
                stage = smallp.tile([HD, DIM], BF, tag="ptstage", bufs=2)
                nc.scalar.copy(stage[:], ps_h[:HD, :DIM])
                r0 = h * HD
                ct0 = r0 // 128
                split = (ct0 + 1) * 128
                if r0 + HD <= split:
                    pieces = [(ct0, r0 - ct0 * 128, 0, HD)]
                else:
                    pieces = [
                        (ct0, r0 - ct0 * 128, 0, split - r0),
                        (ct0 + 1, 0, split - r0, r0 + HD - split),
                    ]
                # engines need 32-aligned partition bases; DMA does not.
                for ct, row0, joff, jlen in pieces:
                    nc.sync.dma_start(
                        PT_sb[ct][row0 : row0 + jlen, :],
                        stage[joff : joff + jlen, :],
                    )

            # ---- phase 3: fused (proj @ A) @ v_dw GEMM -------------------
            VCH = 2048  # v reload chunk

            ngrp = NLOC // VCH
            pending = list(v_pre)  # FIFO of in-flight v chunk groups
            for nt in range(NLOC // 512):
                if nt % (VCH // 512) == 0:
                    g = nt // (VCH // 512)
                    v_sb = pending.pop(0)
                    gn = g + 2  # keep 2 groups in flight (0,1 preloaded)
                    if gn < ngrp:
                        grp = []
                        for t in range(OT_V):
                            vt_ = vinp.tile([128, VCH], BF, tag=f"vin{t}",
                                            name=f"vg{gn}_{t}")
                            nc.sync.dma_start(
                                vt_[:],
                                v_dram[t][:, gn * VCH : (gn + 1) * VCH],
                            )
                            grp.append(vt_)
                        pending.append(grp)
                off = (nt % (VCH // 512)) * 512
                for po in range(CT):
                    ps = psA.tile([128, 512], F32, tag="gemm", name=f"y{nt}_{po}")
                    for t in range(OT_V):
                        nc.tensor.matmul(
                            ps[:, :512],
                            lhsT=PT_sb[t][:, po * 128 : (po + 1) * 128],
                            rhs=v_sb[t][:, off : off + 512],
                            start=(t == 0),
                            stop=(t == OT_V - 1),
                        )
                    ysb = youtp.tile([128, 512], F32, tag="ysb", name=f"ys{nt}_{po}", bufs=4)
                    if po % 2 == 0:
                        nc.scalar.copy(ysb[:], ps[:, :512])
                        nc.scalar.dma_start(
                            y[po][:, nt * 512 : (nt + 1) * 512], ysb[:]
                        )
                    else:
                        nc.vector.tensor_copy(ysb[:], ps[:, :512])
                        nc.gpsimd.dma_start(
                            y[po][:, nt * 512 : (nt + 1) * 512], ysb[:]
                        )

    nc.compile()
    return nc


_NC = None


def _get_program():
    global _NC
    if _NC is None:
        _NC = _build_program()
    return _NC


def _prep_weights(qkv_w, dw_w, proj_w, log_temp):
    """Host-side weight permutation/padding. Returns dict of shared inputs."""
    qkv_w = np.asarray(qkv_w, np.float32)
    dw_w = np.asarray(dw_w, np.float32).reshape(3 * DIM, 9)
    proj_w = np.asarray(proj_w, np.float32)
    temp = np.log1p(np.exp(np.asarray(log_temp, np.float32).reshape(HEADS)))
    temp = temp + 1e-6

    # permutation: first 768 = per head [q_h | k_h]; then v in natural order
    perm = np.concatenate(
        [
            np.concatenate([np.arange(h * HD, (h + 1) * HD),
                            DIM + np.arange(h * HD, (h + 1) * HD)])
            for h in range(HEADS)
        ]
        + [2 * DIM + np.arange(DIM)]
    )
    wq = qkv_w[perm].copy()
    wd = dw_w[perm].copy()
    # prescale qk 1x1 weights into fp8's range; undo in the dw taps
    wq[:CQK] *= QKSCALE
    wd[:CQK] /= QKSCALE

    wqkvT = np.ascontiguousarray(wq.T.reshape(CT, 128, COUT)).astype(BF16)
    wdw = np.ascontiguousarray(wd.reshape(OT, 128, 9))

    # fp8 DoubleRow stationary: wq8[k, j, m] = wq[m, 128j + k], m < 768
    wq8 = np.zeros((128, 2, CQK), np.float32)
    for j in range(2):
        wq8[:, j, :] = wq[:CQK, 128 * j : 128 * (j + 1)].T
    wq8 = np.ascontiguousarray(wq8.reshape(128, 2 * CQK)).astype(FP8)


    # wprojH[i, h*384 + o] = proj_w[o, 48h + i]
    wprojH = np.zeros((HD, HEADS * DIM), np.float32)
    for h in range(HEADS):
        wprojH[:, h * DIM : (h + 1) * DIM] = proj_w[:, h * HD : (h + 1) * HD].T
    wprojH = wprojH.astype(BF16)

    tempb = np.broadcast_to(temp[None, :], (HD, HEADS)).copy()
    eyeb = np.eye(128, dtype=np.float32).astype(BF16)
    eyem = np.tile(np.eye(96, dtype=np.float32), (1, 8)).copy()
    mask8 = np.repeat(np.eye(HEADS, dtype=np.float32), HD, axis=1).astype(BF16)
    wdiag = np.zeros((OT, len(PE_TAPS), 128, 128), np.float32)
    for ot in range(OT):
        for i, s in enumerate(PE_TAPS):
            np.fill_diagonal(wdiag[ot, i], wd[ot * 128 : (ot + 1) * 128, s])
    # sbuf layout: [128 part(k), ntaps*128 free(s, m)]
    wdiag = np.ascontiguousarray(wdiag.transpose(0, 2, 1, 3)).reshape(
        OT, 128, len(PE_TAPS) * 128
    ).astype(BF16)
    return {
        "wqkvT": wqkvT,
        "wq8": wq8,
        "wdw": wdw,
        "wprojH": wprojH,
        "tempb": tempb,
        "eyeb": eyeb,
        "eyem": eyem,
        "mask8": mask8,
        "wdiag": wdiag,
    }


def _prep_x(x):
    """Per-core padded x chunks: bf16 [CT, 128, HP*WP] and the fp8
    interleaved copy of c-tiles 0,1 ([128, 2*X8W])."""
    x = np.asarray(x, np.float32)
    chunks = []
    for c in range(N_CORES):
        b, r0 = c // 4, ROWS * (c % 4)
        buf = np.zeros((DIM, HP, WP), np.float32)
        lo, hi = max(r0 - 1, 0), min(r0 + ROWS + 1, HGT)
        buf[:, lo - (r0 - 1) : hi - (r0 - 1), 1 : WID + 1] = x[b, :, lo:hi, :]
        flat = buf.reshape(CT, 128, HP * WP)
        x8 = np.zeros((128, 2, X8W), np.float32)
        x8[:, :, : HP * WP] = flat[:2].transpose(1, 0, 2)
        chunks.append((
            np.ascontiguousarray(flat).astype(BF16),
            np.ascontiguousarray(x8.reshape(128, 2 * X8W)).astype(FP8),
        ))
    return chunks


def _run(x, qkv_w, dw_w, proj_w, log_temp, trace=False):
    nc = _get_program()
    shared = _prep_weights(qkv_w, dw_w, proj_w, log_temp)
    xchunks = _prep_x(x)
    in_maps = [
        {**shared, "xp": xchunks[c][0], "xp8": xchunks[c][1]}
        for c in range(N_CORES)
    ]
    res = run_bass_kernel_spmd(
        nc, in_maps, core_ids=list(range(N_CORES)), trace=trace
    )
    out = np.empty((B, DIM, HGT, WID), np.float32)
    for c in range(N_CORES):
        b, r0 = c // 4, ROWS * (c % 4)
        yc = res.results[c]["y"].reshape(DIM, ROWS, WID)
        out[b, :, r0 : r0 + ROWS, :] = yc
    return out, res


def kernel(x, qkv_w, dw_w, proj_w, log_temp):
    out, _ = _run(x, qkv_w, dw_w, proj_w, log_temp, trace=False)
    return out


# revision 4
# speedup vs baseline: 1.0073x; 1.0011x over previous
"""MDTA (Restormer channel attention) Bass/Tile kernel for 8 Trainium2 cores.

Sharding: spatial. Core c handles batch b=c//4, image rows 64*(c%4) .. +64.
The channel attention Gram G = Q K^T and the L2 norms are sums over spatial
positions, so each core accumulates per-head joint [q|k] 96x96 Gram partials
locally and one tiny (2 x 96 x 768 fp32) AllReduce combines them; the
normalization (F.normalize) is applied afterwards as row/col scaling of G.

v2 changes vs v1:
  - v channels unpadded (384, proj order). COUT 1280 -> 1152 (9 o-tiles).
  - attention A folded into project_out on device: P^T = blockdiag(A)^T-free
    per-head matmuls (lhsT = softmax L slice, rhs = per-head projT), so
    phase 3 is a single GEMM y = P^T.T @ v_dw. No A^T transposes, no
    separate apply pass.
  - depthwise 3x3 split 5 DVE taps / 4 PE diag-matmul taps; scalar engine
    evacuates the PE partial, DVE does the final 2x-mode merge add.
  - q/k 1x1-conv GEMM in fp8 (DoubleRow over c-tiles 0+1, weights
    prescaled by QKSCALE, undone in the dw taps; full-width matmuls only
    -- column-split DoubleRow corrupts PSUM).
  - per-ot GEMM->depthwise interleave on PE so the DVE tap chain + merge
    never stalls on a whole-block GEMM section.
  - gram DMA transposes issued from the (idle) sync engine.

Device channel layout (host pre-permutes all weights to match):
  o-tiles 0..5  (768 ch): per head h: [q_h (48) | k_h (48)] interleaved -> the
                joint per-head Gram block is contiguous, and its diagonal
                gives ssq/ssk for the L2 norms.
  o-tiles 6..8  (384 ch): v in natural (proj input) order.
"""

import sys

for p in ("/opt/trn_rl_repo", "/opt/pypackages"):
    if p not in sys.path:
        sys.path.insert(0, p)

import numpy as np
import ml_dtypes

import concourse.bass as bass
import concourse.mybir as mybir
import concourse.tile as tile
import concourse.bacc as bacc
from concourse.bass_utils import run_bass_kernel_spmd

BF16 = ml_dtypes.bfloat16

B, DIM, HGT, WID = 2, 384, 256, 256
HEADS = 8
HD = DIM // HEADS  # 48
N_CORES = 8
ROWS = HGT // 4  # 64 output rows per core
WP = WID + 2  # 258 padded width
HP = ROWS + 2  # 66 padded rows per core
NBLK = 8  # row super-blocks per core
BR = ROWS // NBLK  # 8 output rows per block
BP = BR + 2  # 10 padded rows per block
BN = BR * WID  # 2048 output cols per block
NLOC = ROWS * WID  # 16384 output cols per core

CQK = 2 * DIM  # 768 interleaved q/k channels
CV = DIM  # 384 v channels (unpadded, proj order)
COUT = CQK + CV  # 1152 total device channels
OT_QK = CQK // 128  # 6
OT_V = CV // 128  # 3
OT = OT_QK + OT_V  # 9
CT = DIM // 128  # 3 x c-tiles
GSTRIDE = WP * BP  # 2580 cols per block GEMM
# GEMM moving chunks: 16B-aligned starts (DoubleRow AP requirement)
GCHUNKS = ((0, 864), (864, 864), (1728, 852))
WDIAG_TAPS = (4, 5, 6, 7, 8)  # diag-matmul slots in wdiag (taps on PE)
# per-ot alternation: even ots run tap 4 on DVE (5 DVE / 4 PE taps),
# odd ots run it on PE (4 DVE / 5 PE taps) -- balances the two engines
X8W = 17040  # HP*WP (17028) padded to %16 for the fp8 interleave stride
X8C = 2592  # per-block fp8 chunk width (2580 padded to %16)

F32 = mybir.dt.float32
BF = mybir.dt.bfloat16
F8 = mybir.dt.float8e4
FP8 = ml_dtypes.float8_e4m3
QKSCALE = 128.0  # qk weight prescale into fp8's good range (undone in dw)


def _build_program():
    nc = bacc.Bacc(
        "TRN2",
        target_bir_lowering=False,
        debug=False,
        num_devices=N_CORES,
    )

    xp = nc.dram_tensor("xp", [CT, 128, HP * WP], BF, kind="ExternalInput")
    xp8 = nc.dram_tensor("xp8", [128, 2 * X8W], F8, kind="ExternalInput")
    wq8 = nc.dram_tensor("wq8", [128, 2 * CQK], F8, kind="ExternalInput")
    wqkvT = nc.dram_tensor("wqkvT", [CT, 128, COUT], BF, kind="ExternalInput")
    wdw = nc.dram_tensor("wdw", [OT, 128, 9], F32, kind="ExternalInput")
    wprojH = nc.dram_tensor("wprojH", [HD, HEADS * DIM], BF, kind="ExternalInput")
    tempb = nc.dram_tensor("tempb", [HD, HEADS], F32, kind="ExternalInput")
    eyeb = nc.dram_tensor("eyeb", [128, 128], BF, kind="ExternalInput")
    eyem = nc.dram_tensor("eyem", [96, 768], F32, kind="ExternalInput")
    mask8 = nc.dram_tensor("mask8", [HEADS, DIM], BF, kind="ExternalInput")
    wdiag = nc.dram_tensor(
        "wdiag", [OT, 128, len(WDIAG_TAPS) * 128], BF, kind="ExternalInput"
    )
    y = nc.dram_tensor("y", [CT, 128, NLOC], F32, kind="ExternalOutput")

    AOP = mybir.AluOpType
    ACT = mybir.ActivationFunctionType

    with tile.TileContext(nc) as tc:
        with (
            tc.tile_pool(name="const", bufs=1) as constp,
            tc.tile_pool(name="xin", bufs=1) as xinp,
            tc.tile_pool(name="pre", bufs=1) as prep,
            tc.tile_pool(name="acc", bufs=1) as accp,
            tc.tile_pool(name="qkc", bufs=1) as qkcp,
            tc.tile_pool(name="vc", bufs=1) as vcp,
            tc.tile_pool(name="qkt", bufs=2) as qktp,
            tc.tile_pool(name="small", bufs=1) as smallp,
            tc.tile_pool(name="vin", bufs=2) as vinp,
            tc.tile_pool(name="yout", bufs=2) as youtp,
            tc.tile_pool(name="psA", bufs=3, space="PSUM") as psA,
            tc.tile_pool(name="psC", bufs=2, space="PSUM") as psC,
            tc.tile_pool(name="psG", bufs=1, space="PSUM") as psG,
            tc.tile_pool(name="dram", bufs=1, space="DRAM") as dramp,
        ):
            # ---- resident constants --------------------------------------
            wq_sb = []
            for ct in range(CT):
                t = constp.tile([128, COUT], BF, tag=f"wq{ct}")
                nc.sync.dma_start(t[:], wqkvT[ct])
                wq_sb.append(t)
            wq8_sb = constp.tile([128, 2 * CQK], F8, tag="wq8")
            nc.sync.dma_start(wq8_sb[:], wq8[:])
            wdw_sb = []
            for ot in range(OT):
                t = constp.tile([128, 9], F32, tag=f"wdw{ot}")
                nc.sync.dma_start(t[:], wdw[ot])
                wdw_sb.append(t)
            wprojH_sb = constp.tile([HD, HEADS * DIM], BF, tag="wprojH")
            nc.sync.dma_start(wprojH_sb[:], wprojH[:])
            tempb_sb = constp.tile([HD, HEADS], F32, tag="tempb")
            nc.sync.dma_start(tempb_sb[:], tempb[:])
            eyeb_sb = constp.tile([128, 128], BF, tag="eyeb")
            nc.sync.dma_start(eyeb_sb[:], eyeb[:])
            eyem_sb = constp.tile([96, 768], F32, tag="eyem")
            nc.sync.dma_start(eyem_sb[:], eyem[:])
            ones_sb = constp.tile([HEADS, HD], BF, tag="ones")
            nc.vector.memset(ones_sb[:], 1.0)
            mask8_sb = constp.tile([HEADS, DIM], BF, tag="mask8")
            nc.sync.dma_start(mask8_sb[:], mask8[:])
            wdiag_sb = []
            for ot in range(OT):
                t = constp.tile([128, len(WDIAG_TAPS) * 128], BF, tag=f"wdiag{ot}")
                nc.sync.dma_start(t[:], wdiag[ot])
                wdiag_sb.append(t)

            v_dram = dramp.tile([OT_V, 128, NLOC], BF)
            qk_dram = dramp.tile([OT_QK, 128, NLOC], BF)
            cc_in = dramp.tile([96, 768], F32)
            cc_out = dramp.tile([96, 768], F32)

            # Gram accumulators: 2 banks x [96, 4*96] (4 heads per bank)
            gram_ps = [
                psG.tile([96, 384], F32, tag=f"g{i}", name=f"gram{i}")
                for i in range(2)
            ]

            # ---- phase 1: stream row blocks ------------------------------
            qkd2 = qk_dram[:, :, :].rearrange("t p n -> (t p) n")

            def _gram_block(kb):
                for half in range(BN // 128):
                    nt = kb * (BN // 128) + half
                    qkT = qktp.tile([128, CQK], BF, tag="qkT")
                    nc.sync.dma_start_transpose(
                        qkT[:], qkd2[:, nt * 128 : (nt + 1) * 128]
                    )
                    first = kb == 0 and half == 0
                    last = kb == NBLK - 1 and half == BN // 128 - 1
                    for h in range(HEADS):
                        nc.tensor.matmul(
                            gram_ps[h // 4][:, (h % 4) * 96 : (h % 4) * 96 + 96],
                            lhsT=qkT[:, h * 96 : h * 96 + 96],
                            rhs=qkT[:, h * 96 : h * 96 + 96],
                            start=first,
                            stop=last,
                            skip_group_check=True,
                        )

            for k in range(NBLK):
                # x rows 8k .. 8k+10 (padded indexing), all 3 c-tiles
                x_sb = []
                for ct in range(CT):
                    t = xinp.tile([128, GSTRIDE], BF, tag=f"x{ct}")
                    nc.sync.dma_start(
                        t[:], xp[ct][:, k * BR * WP : k * BR * WP + GSTRIDE]
                    )
                    x_sb.append(t)
                # fp8 interleaved view of c-tiles 0,1 for the DoubleRow GEMM
                x8_sb = xinp.tile([128, 2 * X8C], F8, tag="x8")
                for j in range(2):
                    nc.sync.dma_start(
                        x8_sb[:, j * X8C : j * X8C + GSTRIDE],
                        xp8[:, j * X8W + k * BR * WP : j * X8W + k * BR * WP + GSTRIDE],
                    )
                x8r = x8_sb[:].rearrange("p (j n) -> p j n", j=2)
                wq8r = wq8_sb[:].rearrange("p (j m) -> p j m", j=2)

                # qkv pointwise GEMM. qk o-tiles: one fp8 DoubleRow matmul
                # (c-tiles 0+1, weights prescaled by QKSCALE; undone in the
                # dw taps) + one bf16 matmul (c-tile 2). v o-tiles: bf16.
                # Chunk starts 16B-aligned (DoubleRow moving-AP rule); a
                # DoubleRow matmul must be full-width (no column splits).
                pre_sb = []
                for ot in range(OT):
                    t = prep.tile([128, GSTRIDE], BF, tag=f"pre{ot}")
                    pre_sb.append(t)
                # per-ot pipeline: GEMM(ot) then depthwise(ot) on PE, so
                # the DVE chain + merge for ot never waits on a whole-block
                # PE GEMM section (kills the phase-1 PE/DVE alternation).
                for ot in range(OT):
                    for g0, glen in ((0, 432), (432, 432), (864, 432),
                                     (1296, 432), (1728, 432), (2160, 420)):
                        ps = psA.tile([128, 512], F32, tag="gemm")
                        if ot < OT_QK:
                            nc.tensor.matmul(
                                ps[:, :glen],
                                lhsT=wq8r[:, :, ot * 128 : (ot + 1) * 128],
                                rhs=x8r[:, :, g0 : g0 + glen],
                                start=True,
                                stop=False,
                                perf_mode=mybir.MatmulPerfMode.DoubleRow,
                            )
                            nc.tensor.matmul(
                                ps[:, :glen],
                                lhsT=wq_sb[2][:, ot * 128 : (ot + 1) * 128],
                                rhs=x_sb[2][:, g0 : g0 + glen],
                                start=False,
                                stop=True,
                            )
                        else:
                            for ct in range(CT):
                                nc.tensor.matmul(
                                    ps[:, :glen],
                                    lhsT=wq_sb[ct][:, ot * 128 : (ot + 1) * 128],
                                    rhs=x_sb[ct][:, g0 : g0 + glen],
                                    start=(ct == 0),
                                    stop=(ct == CT - 1),
                                )
                        nc.scalar.copy(
                            pre_sb[ot][:, g0 : g0 + glen], ps[:, :glen]
                        )

                    # depthwise 3x3 for this ot: DVE_TAPS on DVE, PE_TAPS as
                    # diag-matmul PSUM accumulation; scalar evacuates the PE
                    # partial, DVE merges.
                    pre_r = pre_sb[ot][:].rearrange("p (r w) -> p r w", w=WP)
                    if ot < OT_QK:
                        dst = qkcp.tile([128, BN], BF, tag=f"qk{ot}")
                    else:
                        dst = vcp.tile([128, BN], BF, tag=f"v{ot - OT_QK}")
                    accA = accp.tile([128, BN], BF, tag="accA")
                    accB = accp.tile([128, BN], BF, tag="accB")
                    tmp = accp.tile([128, BN], BF, tag="tmp")
                    pp = accp.tile([128, BN], BF, tag="pp", bufs=2)
                    dve_taps = (0, 1, 2, 3, 4) if ot % 2 == 0 else (0, 1, 2, 3)
                    pe_taps = (5, 6, 7, 8) if ot % 2 == 0 else (4, 5, 6, 7, 8)
                    pair = [accA, accB]
                    for i, s in enumerate(dve_taps):
                        dh, dw = s // 3, s % 3
                        srcap = pre_r[:, dh : dh + BR, dw : dw + WID]
                        wcol = wdw_sb[ot][:, s : s + 1]
                        cur, nxt = pair[(i + 1) % 2], pair[i % 2]
                        if i == 0:
                            nc.vector.tensor_scalar_mul(nxt[:], srcap, wcol)
                        else:
                            nc.vector.tensor_scalar_mul(tmp[:], srcap, wcol)
                            nc.vector.tensor_tensor(
                                nxt[:], cur[:], tmp[:], AOP.add
                            )
                    accD = pair[(len(dve_taps) - 1) % 2]
                    for g in range(4):
                        pc = psC.tile([128, 512], F32, tag="conv")
                        for i, s in enumerate(pe_taps):
                            idx = s - 4  # slot in wdiag (taps 4..8)
                            dh, dw = s // 3, s % 3
                            rhs = pre_r[
                                :, dh + 2 * g : dh + 2 * g + 2, dw : dw + WID
                            ]
                            nc.tensor.matmul(
                                pc[:],
                                lhsT=wdiag_sb[ot][:, idx * 128 : (idx + 1) * 128],
                                rhs=rhs,
                                start=(i == 0),
                                stop=(i == len(pe_taps) - 1),
                            )
                        nc.scalar.copy(pp[:, g * 512 : (g + 1) * 512], pc[:])
                    nc.vector.tensor_tensor(dst[:], accD[:], pp[:], AOP.add)
                    if ot >= OT_QK:
                        vt = ot - OT_QK
                        nc.sync.dma_start(
                            v_dram[vt][:, k * BN : (k + 1) * BN], dst[:]
                        )
                    else:
                        nc.sync.dma_start(
                            qk_dram[ot][:, k * BN : (k + 1) * BN], dst[:]
                        )

                # xbar-transpose q/k n-tiles from DRAM, accumulate Grams.
                # Lagged one block (process block k-1 here, block NBLK-1
                # after the loop) so the gram matmuls never stall PE on the
                # merge->DMA->transpose chain of the current block.
                if k > 0:
                    _gram_block(k - 1)

            _gram_block(NBLK - 1)

            # ---- phase 1.5: per-batch-group AllReduce --------------------
            # prefetch phase-3's first two v chunks behind the collective
            v_pre = []
            for g in range(2):
                grp = []
                for t in range(OT_V):
                    vt_ = vinp.tile([128, 1024], BF, tag=f"vin{t}",
                                    name=f"vpre{g}_{t}")
                    nc.scalar.dma_start(
                        vt_[:], v_dram[t][:, g * 1024 : (g + 1) * 1024]
                    )
                    grp.append(vt_)
                v_pre.append(grp)
            ccin_sb = smallp.tile([96, 768], F32, tag="ccin")
            for g in range(2):
                nc.scalar.copy(
                    ccin_sb[:, g * 384 : (g + 1) * 384], gram_ps[g][:]
                )
            nc.sync.dma_start(cc_in[:], ccin_sb[:])
            nc.gpsimd.collective_compute(
                "AllReduce",
                AOP.add,
                replica_groups=[[0, 1, 2, 3], [4, 5, 6, 7]],
                ins=[cc_in.opt()],
                outs=[cc_out.opt()],
            )
            gred = smallp.tile([96, 768], F32, tag="gred")
            nc.sync.dma_start(gred[:], cc_out[:])

            # ---- phase 2: norms, scaling, softmax ------------------------
            # diag -> per-channel sum of squares [96(joint c), 8(head)]
            dm = smallp.tile([96, 768], F32, tag="dm")
            nc.vector.tensor_tensor(dm[:], gred[:], eyem_sb[:], AOP.mult)
            dsum = smallp.tile([96, HEADS], F32, tag="dsum")
            nc.vector.tensor_reduce(
                dsum[:],
                dm[:].rearrange("p (h d) -> p h d", d=96),
                axis=mybir.AxisListType.X,
                op=AOP.add,
            )
            norms = smallp.tile([96, HEADS], F32, tag="norms")
            nc.scalar.sqrt(norms[:], dsum[:])
            nc.vector.tensor_scalar_max(norms[:], norms[:], 1e-12)
            rsc = smallp.tile([96, HEADS], F32, tag="rsc")
            nc.vector.reciprocal(rsc[:], norms[:])

            # rk broadcast [48, h*48+d] = rsc[48+d, h]:
            # transpose rsc -> rscT [8, 96]; rkrep[h', (h,d)] = rscT[h', 48+d]
            # masked by delta(h'=h); then ones[8,48].T @ rkrep sums out h'.
            rscb = smallp.tile([96, HEADS], BF, tag="rscb")
            nc.vector.tensor_copy(rscb[:], rsc[:])
            rscT_ps = psC.tile([128, 128], BF, tag="conv")
            nc.tensor.transpose(
                rscT_ps[:HEADS, :96], rscb[:], eyeb_sb[:96, :96]
            )
            rscT = smallp.tile([HEADS, 96], BF, tag="rscT")
            nc.vector.tensor_copy(rscT[:], rscT_ps[:HEADS, :96])
            rkrep = smallp.tile([HEADS, DIM], BF, tag="rkrep")
            mask3d = mask8_sb[:].rearrange("p (h d) -> p h d", d=HD)
            rk3d = rscT[:, HD : 2 * HD].rearrange("p (o d) -> p o d", o=1)
            mask3d, rk3d = bass.broadcast_tensor_aps(mask3d, rk3d)
            nc.vector.tensor_tensor(
                rkrep[:].rearrange("p (h d) -> p h d", d=HD),
                mask3d,
                rk3d,
                AOP.mult,
            )
            rkb_ps = psA.tile([128, 512], F32, tag="gemm")
            nc.tensor.matmul(
                rkb_ps[:HD, :DIM],
                lhsT=ones_sb[:],
                rhs=rkrep[:],
                start=True,
                stop=True,
            )
            # logits L[c, h, d] = G_qk * rk * (temp_h * rq)
            L = smallp.tile([HD, DIM], F32, tag="L")
            gqk = gred[0:HD].rearrange("p (h d) -> p h d", d=96)[:, :, HD : 2 * HD]
            nc.vector.tensor_tensor(
                L[:].rearrange("p (h d) -> p h d", d=HD),
                gqk,
                rkb_ps[:HD, :DIM].rearrange("p (h d) -> p h d", d=HD),
                AOP.mult,
            )
            tsc = smallp.tile([HD, HEADS], F32, tag="tsc")
            nc.vector.tensor_tensor(tsc[:], tempb_sb[:], rsc[0:HD, :], AOP.mult)
            for h in range(HEADS):
                nc.vector.tensor_scalar_mul(
                    L[:, h * HD : (h + 1) * HD],
                    L[:, h * HD : (h + 1) * HD],
                    tsc[:, h : h + 1],
                )
            # softmax over d (free dim, per 48-block)
            mx = smallp.tile([HD, HEADS], F32, tag="mx")
            nc.vector.tensor_reduce(
                mx[:],
                L[:].rearrange("p (h d) -> p h d", d=HD),
                axis=mybir.AxisListType.X,
                op=AOP.max,
            )
            for h in range(HEADS):
                nc.vector.tensor_scalar_sub(
                    L[:, h * HD : (h + 1) * HD],
                    L[:, h * HD : (h + 1) * HD],
                    mx[:, h : h + 1],
                )
            nc.scalar.activation(L[:], L[:], ACT.Exp)
            sm = smallp.tile([HD, HEADS], F32, tag="sm")
            nc.vector.tensor_reduce(
                sm[:],
                L[:].rearrange("p (h d) -> p h d", d=HD),
                axis=mybir.AxisListType.X,
                op=AOP.add,
            )
            rs = smallp.tile([HD, HEADS], F32, tag="rs")
            nc.vector.reciprocal(rs[:], sm[:])
            for h in range(HEADS):
                nc.vector.tensor_scalar_mul(
                    L[:, h * HD : (h + 1) * HD],
                    L[:, h * HD : (h + 1) * HD],
                    rs[:, h : h + 1],
                )
            Lb = smallp.tile([HD, DIM], BF, tag="Lb")
            nc.vector.tensor_copy(Lb[:], L[:])

            # ---- phase 2.5: fold A into proj: P^T[48h+j, o] =
            # sum_i A_h[i,j] * proj[o, 48h+i]; lhsT = Lb[:, h*48+j] slice,
            # rhs = wprojH[:, h*384 : (h+1)*384]. Each head computed at
            # partition 0 in PSUM, then scalar-copied into the P^T tiles
            # (split where a head straddles a 128-partition boundary).
            PT_sb = []
            for ct in range(CT):
                t = smallp.tile([128, DIM], BF, tag=f"PT{ct}")
                PT_sb.append(t)
            for h in range(HEADS):
                ps_h = psA.tile([128, 512], F32, tag="gemm", name=f"ptps{h}")
                nc.tensor.matmul(
                    ps_h[:HD, :DIM],
                    lhsT=Lb[:, h * HD : (h + 1) * HD],
                    rhs=wprojH_sb[:, h * DIM : (h + 1) * DIM],
                    start=True,
                    stop=True,
                )
                stage = smallp.tile([HD, DIM], BF, tag="ptstage", bufs=2)
                nc.scalar.copy(stage[:], ps_h[:HD, :DIM])
                r0 = h * HD
                ct0 = r0 // 128
                split = (ct0 + 1) * 128
                if r0 + HD <= split:
                    pieces = [(ct0, r0 - ct0 * 128, 0, HD)]
                else:
                    pieces = [
                        (ct0, r0 - ct0 * 128, 0, split - r0),
                        (ct0 + 1, 0, split - r0, r0 + HD - split),
                    ]
                # engines need 32-aligned partition bases; DMA does not.
                for ct, row0, joff, jlen in pieces:
                    nc.sync.dma_start(
                        PT_sb[ct][row0 : row0 + jlen, :],
                        stage[joff : joff + jlen, :],
                    )

            # ---- phase 3: fused (proj @ A) @ v_dw GEMM -------------------
            VCH = 1024  # v reload chunk

            ngrp = NLOC // VCH
            pending = list(v_pre)  # FIFO of in-flight v chunk groups
            for nt in range(NLOC // 512):
                if nt % (VCH // 512) == 0:
                    g = nt // (VCH // 512)
                    v_sb = pending.pop(0)
                    gn = g + 2  # keep 2 groups in flight (0,1 preloaded)
                    if gn < ngrp:
                        grp = []
                        for t in range(OT_V):
                            vt_ = vinp.tile([128, VCH], BF, tag=f"vin{t}",
                                            name=f"vg{gn}_{t}")
                            nc.sync.dma_start(
                                vt_[:],
                                v_dram[t][:, gn * VCH : (gn + 1) * VCH],
                            )
                            grp.append(vt_)
                        pending.append(grp)
                off = (nt % (VCH // 512)) * 512
                for po in range(CT):
                    ps = psA.tile([128, 512], F32, tag="gemm", name=f"y{nt}_{po}")
                    for t in range(OT_V):
                        nc.tensor.matmul(
                            ps[:, :512],
                            lhsT=PT_sb[t][:, po * 128 : (po + 1) * 128],
                            rhs=v_sb[t][:, off : off + 512],
                            start=(t == 0),
                            stop=(t == OT_V - 1),
                        )
                    ysb = youtp.tile([128, 512], F32, tag="ysb", name=f"ys{nt}_{po}", bufs=4)
                    if po % 2 == 0:
                        nc.scalar.copy(ysb[:], ps[:, :512])
                        nc.scalar.dma_start(
                            y[po][:, nt * 512 : (nt + 1) * 512], ysb[:]
                        )
                    else:
                        nc.vector.tensor_copy(ysb[:], ps[:, :512])
                        nc.gpsimd.dma_start(
                            y[po][:, nt * 512 : (nt + 1) * 512], ysb[:]
                        )

    nc.compile()
    return nc


_NC = None


def _get_program():
    global _NC
    if _NC is None:
        _NC = _build_program()
    return _NC


def _prep_weights(qkv_w, dw_w, proj_w, log_temp):
    """Host-side weight permutation/padding. Returns dict of shared inputs."""
    qkv_w = np.asarray(qkv_w, np.float32)
    dw_w = np.asarray(dw_w, np.float32).reshape(3 * DIM, 9)
    proj_w = np.asarray(proj_w, np.float32)
    temp = np.log1p(np.exp(np.asarray(log_temp, np.float32).reshape(HEADS)))
    temp = temp + 1e-6

    # permutation: first 768 = per head [q_h | k_h]; then v in natural order
    perm = np.concatenate(
        [
            np.concatenate([np.arange(h * HD, (h + 1) * HD),
                            DIM + np.arange(h * HD, (h + 1) * HD)])
            for h in range(HEADS)
        ]
        + [2 * DIM + np.arange(DIM)]
    )
    wq = qkv_w[perm].copy()
    wd = dw_w[perm].copy()
    # prescale qk 1x1 weights into fp8's range; undo in the dw taps
    wq[:CQK] *= QKSCALE
    wd[:CQK] /= QKSCALE

    wqkvT = np.ascontiguousarray(wq.T.reshape(CT, 128, COUT)).astype(BF16)
    wdw = np.ascontiguousarray(wd.reshape(OT, 128, 9))

    # fp8 DoubleRow stationary: wq8[k, j, m] = wq[m, 128j + k], m < 768
    wq8 = np.zeros((128, 2, CQK), np.float32)
    for j in range(2):
        wq8[:, j, :] = wq[:CQK, 128 * j : 128 * (j + 1)].T
    wq8 = np.ascontiguousarray(wq8.reshape(128, 2 * CQK)).astype(FP8)


    # wprojH[i, h*384 + o] = proj_w[o, 48h + i]
    wprojH = np.zeros((HD, HEADS * DIM), np.float32)
    for h in range(HEADS):
        wprojH[:, h * DIM : (h + 1) * DIM] = proj_w[:, h * HD : (h + 1) * HD].T
    wprojH = wprojH.astype(BF16)

    tempb = np.broadcast_to(temp[None, :], (HD, HEADS)).copy()
    eyeb = np.eye(128, dtype=np.float32).astype(BF16)
    eyem = np.tile(np.eye(96, dtype=np.float32), (1, 8)).copy()
    mask8 = np.repeat(np.eye(HEADS, dtype=np.float32), HD, axis=1).astype(BF16)
    wdiag = np.zeros((OT, len(WDIAG_TAPS), 128, 128), np.float32)
    for ot in range(OT):
        for i, s in enumerate(WDIAG_TAPS):
            np.fill_diagonal(wdiag[ot, i], wd[ot * 128 : (ot + 1) * 128, s])
    # sbuf layout: [128 part(k), ntaps*128 free(s, m)]
    wdiag = np.ascontiguousarray(wdiag.transpose(0, 2, 1, 3)).reshape(
        OT, 128, len(WDIAG_TAPS) * 128
    ).astype(BF16)
    return {
        "wqkvT": wqkvT,
        "wq8": wq8,
        "wdw": wdw,
        "wprojH": wprojH,
        "tempb": tempb,
        "eyeb": eyeb,
        "eyem": eyem,
        "mask8": mask8,
        "wdiag": wdiag,
    }


def _prep_x(x):
    """Per-core padded x chunks: bf16 [CT, 128, HP*WP] and the fp8
    interleaved copy of c-tiles 0,1 ([128, 2*X8W])."""
    x = np.asarray(x, np.float32)
    chunks = []
    for c in range(N_CORES):
        b, r0 = c // 4, ROWS * (c % 4)
        buf = np.zeros((DIM, HP, WP), np.float32)
        lo, hi = max(r0 - 1, 0), min(r0 + ROWS + 1, HGT)
        buf[:, lo - (r0 - 1) : hi - (r0 - 1), 1 : WID + 1] = x[b, :, lo:hi, :]
        flat = buf.reshape(CT, 128, HP * WP)
        x8 = np.zeros((128, 2, X8W), np.float32)
        x8[:, :, : HP * WP] = flat[:2].transpose(1, 0, 2)
        chunks.append((
            np.ascontiguousarray(flat).astype(BF16),
            np.ascontiguousarray(x8.reshape(128, 2 * X8W)).astype(FP8),
        ))
    return chunks


def _run(x, qkv_w, dw_w, proj_w, log_temp, trace=False):
    nc = _get_program()
    shared = _prep_weights(qkv_w, dw_w, proj_w, log_temp)
    xchunks = _prep_x(x)
    in_maps = [
        {**shared, "xp": xchunks[c][0], "xp8": xchunks[c][1]}
        for c in range(N_CORES)
    ]
    res = run_bass_kernel_spmd(
        nc, in_maps, core_ids=list(range(N_CORES)), trace=trace
    )
    out = np.empty((B, DIM, HGT, WID), np.float32)
    for c in range(N_CORES):
        b, r0 = c // 4, ROWS * (c % 4)
        yc = res.results[c]["y"].reshape(DIM, ROWS, WID)
        out[b, :, r0 : r0 + ROWS, :] = yc
    return out, res


def kernel(x, qkv_w, dw_w, proj_w, log_temp):
    out, _ = _run(x, qkv_w, dw_w, proj_w, log_temp, trace=False)
    return out


# revision 5
# speedup vs baseline: 1.0421x; 1.0345x over previous
"""MDTA (Restormer channel attention) Bass/Tile kernel for 8 Trainium2 cores.

Sharding: spatial. Core c handles batch b=c//4, image rows 64*(c%4) .. +64.
The channel attention Gram G = Q K^T and the L2 norms are sums over spatial
positions, so each core accumulates per-head joint [q|k] 96x96 Gram partials
locally and one tiny (2 x 96 x 768 fp32) AllReduce combines them; the
normalization (F.normalize) is applied afterwards as row/col scaling of G.

v2 changes vs v1:
  - v channels unpadded (384, proj order). COUT 1280 -> 1152 (9 o-tiles).
  - attention A folded into project_out on device: P^T = blockdiag(A)^T-free
    per-head matmuls (lhsT = softmax L slice, rhs = per-head projT), so
    phase 3 is a single GEMM y = P^T.T @ v_dw. No A^T transposes, no
    separate apply pass.
  - depthwise 3x3 split 5 DVE taps / 4 PE diag-matmul taps; scalar engine
    evacuates the PE partial, DVE does the final 2x-mode merge add.
  - q/k 1x1-conv GEMM in fp8 (DoubleRow over c-tiles 0+1, weights
    prescaled by QKSCALE, undone in the dw taps; full-width matmuls only
    -- column-split DoubleRow corrupts PSUM).
  - per-ot GEMM->depthwise interleave on PE so the DVE tap chain + merge
    never stalls on a whole-block GEMM section.
  - gram DMA transposes issued from the (idle) sync engine.

Device channel layout (host pre-permutes all weights to match):
  o-tiles 0..5  (768 ch): per head h: [q_h (48) | k_h (48)] interleaved -> the
                joint per-head Gram block is contiguous, and its diagonal
                gives ssq/ssk for the L2 norms.
  o-tiles 6..8  (384 ch): v in natural (proj input) order.
"""

import sys

for p in ("/opt/trn_rl_repo", "/opt/pypackages"):
    if p not in sys.path:
        sys.path.insert(0, p)

import numpy as np
import ml_dtypes

import concourse.bass as bass
import concourse.mybir as mybir
import concourse.tile as tile
import concourse.bacc as bacc
from concourse.bass_utils import run_bass_kernel_spmd

BF16 = ml_dtypes.bfloat16

B, DIM, HGT, WID = 2, 384, 256, 256
HEADS = 8
HD = DIM // HEADS  # 48
N_CORES = 8
ROWS = HGT // 4  # 64 output rows per core
WP = WID + 2  # 258 padded width
HP = ROWS + 2  # 66 padded rows per core
NBLK = 8  # row super-blocks per core
BR = ROWS // NBLK  # 8 output rows per block
BP = BR + 2  # 10 padded rows per block
BN = BR * WID  # 2048 output cols per block
NLOC = ROWS * WID  # 16384 output cols per core

CQK = 2 * DIM  # 768 interleaved q/k channels
CV = DIM  # 384 v channels (unpadded, proj order)
COUT = CQK + CV  # 1152 total device channels
OT_QK = CQK // 128  # 6
OT_V = CV // 128  # 3
OT = OT_QK + OT_V  # 9
CT = DIM // 128  # 3 x c-tiles
GSTRIDE = WP * BP  # 2580 cols per block GEMM
# GEMM moving chunks: 16B-aligned starts (DoubleRow AP requirement)
GCHUNKS = ((0, 864), (864, 864), (1728, 852))
WDIAG_TAPS = (4, 5, 6, 7, 8)  # diag-matmul slots in wdiag (taps on PE)
# per-ot alternation: even ots run tap 4 on DVE (5 DVE / 4 PE taps),
# odd ots run it on PE (4 DVE / 5 PE taps) -- balances the two engines
X8W = 17040  # HP*WP (17028) padded to %16 for the fp8 interleave stride
X8C = 2592  # per-block fp8 chunk width (2580 padded to %16)

F32 = mybir.dt.float32
BF = mybir.dt.bfloat16
F8 = mybir.dt.float8e4
FP8 = ml_dtypes.float8_e4m3
QKSCALE = 128.0  # qk weight prescale into fp8's good range (undone in dw)


def _build_program():
    nc = bacc.Bacc(
        "TRN2",
        target_bir_lowering=False,
        debug=False,
        num_devices=N_CORES,
    )

    xp = nc.dram_tensor("xp", [CT, 128, HP * WP], BF, kind="ExternalInput")
    xp8 = nc.dram_tensor("xp8", [128, 2 * X8W], F8, kind="ExternalInput")
    wq8 = nc.dram_tensor("wq8", [128, 2 * CQK], F8, kind="ExternalInput")
    wqkvT = nc.dram_tensor("wqkvT", [CT, 128, COUT], BF, kind="ExternalInput")
    wdw = nc.dram_tensor("wdw", [OT, 128, 9], F32, kind="ExternalInput")
    wprojH = nc.dram_tensor("wprojH", [HD, HEADS * DIM], BF, kind="ExternalInput")
    tempb = nc.dram_tensor("tempb", [HD, HEADS], F32, kind="ExternalInput")
    eyeb = nc.dram_tensor("eyeb", [128, 128], BF, kind="ExternalInput")
    eyem = nc.dram_tensor("eyem", [96, 768], F32, kind="ExternalInput")
    mask8 = nc.dram_tensor("mask8", [HEADS, DIM], BF, kind="ExternalInput")
    wdiag = nc.dram_tensor(
        "wdiag", [OT, 128, len(WDIAG_TAPS) * 128], BF, kind="ExternalInput"
    )
    y = nc.dram_tensor("y", [CT, 128, NLOC], F32, kind="ExternalOutput")

    AOP = mybir.AluOpType
    ACT = mybir.ActivationFunctionType

    with tile.TileContext(nc) as tc:
        with (
            tc.tile_pool(name="const", bufs=1) as constp,
            tc.tile_pool(name="xin", bufs=1) as xinp,
            tc.tile_pool(name="pre", bufs=1) as prep,
            tc.tile_pool(name="acc", bufs=1) as accp,
            tc.tile_pool(name="qkc", bufs=1) as qkcp,
            tc.tile_pool(name="vc", bufs=1) as vcp,
            tc.tile_pool(name="qkt", bufs=2) as qktp,
            tc.tile_pool(name="small", bufs=1) as smallp,
            tc.tile_pool(name="vin", bufs=2) as vinp,
            tc.tile_pool(name="yout", bufs=2) as youtp,
            tc.tile_pool(name="psA", bufs=3, space="PSUM") as psA,
            tc.tile_pool(name="psC", bufs=2, space="PSUM") as psC,
            tc.tile_pool(name="psG", bufs=1, space="PSUM") as psG,
            tc.tile_pool(name="dram", bufs=1, space="DRAM") as dramp,
        ):
            # ---- resident constants --------------------------------------
            wq_sb = []
            for ct in range(CT):
                t = constp.tile([128, COUT], BF, tag=f"wq{ct}")
                nc.sync.dma_start(t[:], wqkvT[ct])
                wq_sb.append(t)
            wq8_sb = constp.tile([128, 2 * CQK], F8, tag="wq8")
            nc.sync.dma_start(wq8_sb[:], wq8[:])
            wdw_sb = []
            for ot in range(OT):
                t = constp.tile([128, 9], F32, tag=f"wdw{ot}")
                nc.sync.dma_start(t[:], wdw[ot])
                wdw_sb.append(t)
            wprojH_sb = constp.tile([HD, HEADS * DIM], BF, tag="wprojH")
            nc.sync.dma_start(wprojH_sb[:], wprojH[:])
            tempb_sb = constp.tile([HD, HEADS], F32, tag="tempb")
            nc.sync.dma_start(tempb_sb[:], tempb[:])
            eyeb_sb = constp.tile([128, 128], BF, tag="eyeb")
            nc.sync.dma_start(eyeb_sb[:], eyeb[:])
            eyem_sb = constp.tile([96, 768], F32, tag="eyem")
            nc.sync.dma_start(eyem_sb[:], eyem[:])
            ones_sb = constp.tile([HEADS, HD], BF, tag="ones")
            nc.vector.memset(ones_sb[:], 1.0)
            mask8_sb = constp.tile([HEADS, DIM], BF, tag="mask8")
            nc.sync.dma_start(mask8_sb[:], mask8[:])
            wdiag_sb = []
            for ot in range(OT):
                t = constp.tile([128, len(WDIAG_TAPS) * 128], BF, tag=f"wdiag{ot}")
                nc.sync.dma_start(t[:], wdiag[ot])
                wdiag_sb.append(t)

            v_dram = dramp.tile([OT_V, 128, NLOC], BF)
            qk_dram = dramp.tile([OT_QK, 128, NLOC], BF)
            cc_in = dramp.tile([96, 768], F32)
            cc_out = dramp.tile([96, 768], F32)

            # Gram accumulators: 2 banks x [96, 4*96] (4 heads per bank)
            gram_ps = [
                psG.tile([96, 384], F32, tag=f"g{i}", name=f"gram{i}")
                for i in range(2)
            ]

            # ---- phase 1: stream row blocks ------------------------------
            qkd2 = qk_dram[:, :, :].rearrange("t p n -> (t p) n")

            def _gram_block(kb):
                for half in range(BN // 128):
                    nt = kb * (BN // 128) + half
                    qkT = qktp.tile([128, CQK], BF, tag="qkT")
                    nc.sync.dma_start_transpose(
                        qkT[:], qkd2[:, nt * 128 : (nt + 1) * 128]
                    )
                    first = kb == 0 and half == 0
                    last = kb == NBLK - 1 and half == BN // 128 - 1
                    for h in range(HEADS):
                        nc.tensor.matmul(
                            gram_ps[h // 4][:, (h % 4) * 96 : (h % 4) * 96 + 96],
                            lhsT=qkT[:, h * 96 : h * 96 + 96],
                            rhs=qkT[:, h * 96 : h * 96 + 96],
                            start=first,
                            stop=last,
                            skip_group_check=True,
                        )

            for k in range(NBLK):
                # x rows 8k .. 8k+10 (padded indexing), all 3 c-tiles
                x_sb = []
                for ct in range(CT):
                    t = xinp.tile([128, GSTRIDE], BF, tag=f"x{ct}")
                    nc.sync.dma_start(
                        t[:], xp[ct][:, k * BR * WP : k * BR * WP + GSTRIDE]
                    )
                    x_sb.append(t)
                # fp8 interleaved view of c-tiles 0,1 for the DoubleRow GEMM
                x8_sb = xinp.tile([128, 2 * X8C], F8, tag="x8")
                for j in range(2):
                    nc.sync.dma_start(
                        x8_sb[:, j * X8C : j * X8C + GSTRIDE],
                        xp8[:, j * X8W + k * BR * WP : j * X8W + k * BR * WP + GSTRIDE],
                    )
                x8r = x8_sb[:].rearrange("p (j n) -> p j n", j=2)
                wq8r = wq8_sb[:].rearrange("p (j m) -> p j m", j=2)

                # qkv pointwise GEMM. qk o-tiles: one fp8 DoubleRow matmul
                # (c-tiles 0+1, weights prescaled by QKSCALE; undone in the
                # dw taps) + one bf16 matmul (c-tile 2). v o-tiles: bf16.
                # Chunk starts 16B-aligned (DoubleRow moving-AP rule); a
                # DoubleRow matmul must be full-width (no column splits).
                pre_sb = []
                for ot in range(OT):
                    t = prep.tile([128, GSTRIDE], BF, tag=f"pre{ot}",
                                  name=f"pre{k}_{ot}")
                    pre_sb.append(t)
                if k == 0:
                    gchunks = ((0, 432), (432, 432), (864, 432),
                               (1296, 432), (1728, 432), (2160, 420))
                else:
                    # rolling halo: rows 8k-1,8k (cols 0..512) are copied
                    # from the previous block's tail (same SBUF slot, so
                    # tap APs stay contiguous); GEMM recomputes only cols
                    # 512..2580. Chunk starts stay 16-aligned for DoubleRow.
                    gchunks = ((512, 432), (944, 432), (1376, 432),
                               (1808, 432), (2240, 340))
                # per-ot pipeline: GEMM(ot) then depthwise(ot) on PE, so
                # the DVE chain + merge for ot never waits on a whole-block
                # PE GEMM section (kills the phase-1 PE/DVE alternation).
                for ot in range(OT):
                    if k > 0:
                        nc.scalar.copy(
                            pre_sb[ot][:, 0:512],
                            pre_sb[ot][:, 2064:2576],
                        )
                    for g0, glen in gchunks:
                        ps = psA.tile([128, 512], F32, tag="gemm")
                        if ot < OT_QK:
                            nc.tensor.matmul(
                                ps[:, :glen],
                                lhsT=wq8r[:, :, ot * 128 : (ot + 1) * 128],
                                rhs=x8r[:, :, g0 : g0 + glen],
                                start=True,
                                stop=False,
                                perf_mode=mybir.MatmulPerfMode.DoubleRow,
                            )
                            nc.tensor.matmul(
                                ps[:, :glen],
                                lhsT=wq_sb[2][:, ot * 128 : (ot + 1) * 128],
                                rhs=x_sb[2][:, g0 : g0 + glen],
                                start=False,
                                stop=True,
                            )
                        else:
                            for ct in range(CT):
                                nc.tensor.matmul(
                                    ps[:, :glen],
                                    lhsT=wq_sb[ct][:, ot * 128 : (ot + 1) * 128],
                                    rhs=x_sb[ct][:, g0 : g0 + glen],
                                    start=(ct == 0),
                                    stop=(ct == CT - 1),
                                )
                        nc.scalar.copy(
                            pre_sb[ot][:, g0 : g0 + glen], ps[:, :glen]
                        )

                    # depthwise 3x3 for this ot: DVE_TAPS on DVE, PE_TAPS as
                    # diag-matmul PSUM accumulation; scalar evacuates the PE
                    # partial, DVE merges.
                    pre_r = pre_sb[ot][:].rearrange("p (r w) -> p r w", w=WP)
                    if ot < OT_QK:
                        dst = qkcp.tile([128, BN], BF, tag=f"qk{ot}")
                    else:
                        dst = vcp.tile([128, BN], BF, tag=f"v{ot - OT_QK}")
                    accA = accp.tile([128, BN], BF, tag="accA")
                    accB = accp.tile([128, BN], BF, tag="accB")
                    tmp = accp.tile([128, BN], BF, tag="tmp")
                    pp = accp.tile([128, BN], BF, tag="pp", bufs=2)
                    dve_taps = (0, 1, 2, 3, 4) if ot % 2 == 0 else (0, 1, 2, 3)
                    pe_taps = (5, 6, 7, 8) if ot % 2 == 0 else (4, 5, 6, 7, 8)
                    pair = [accA, accB]
                    for i, s in enumerate(dve_taps):
                        dh, dw = s // 3, s % 3
                        srcap = pre_r[:, dh : dh + BR, dw : dw + WID]
                        wcol = wdw_sb[ot][:, s : s + 1]
                        cur, nxt = pair[(i + 1) % 2], pair[i % 2]
                        if i == 0:
                            nc.vector.tensor_scalar_mul(nxt[:], srcap, wcol)
                        else:
                            nc.vector.tensor_scalar_mul(tmp[:], srcap, wcol)
                            nc.vector.tensor_tensor(
                                nxt[:], cur[:], tmp[:], AOP.add
                            )
                    accD = pair[(len(dve_taps) - 1) % 2]
                    for g in range(4):
                        pc = psC.tile([128, 512], F32, tag="conv")
                        for i, s in enumerate(pe_taps):
                            idx = s - 4  # slot in wdiag (taps 4..8)
                            dh, dw = s // 3, s % 3
                            rhs = pre_r[
                                :, dh + 2 * g : dh + 2 * g + 2, dw : dw + WID
                            ]
                            nc.tensor.matmul(
                                pc[:],
                                lhsT=wdiag_sb[ot][:, idx * 128 : (idx + 1) * 128],
                                rhs=rhs,
                                start=(i == 0),
                                stop=(i == len(pe_taps) - 1),
                            )
                        nc.scalar.copy(pp[:, g * 512 : (g + 1) * 512], pc[:])
                    nc.vector.tensor_tensor(dst[:], accD[:], pp[:], AOP.add)
                    if ot >= OT_QK:
                        vt = ot - OT_QK
                        nc.sync.dma_start(
                            v_dram[vt][:, k * BN : (k + 1) * BN], dst[:]
                        )
                    else:
                        nc.sync.dma_start(
                            qk_dram[ot][:, k * BN : (k + 1) * BN], dst[:]
                        )

                prev_pre = pre_sb

                # xbar-transpose q/k n-tiles from DRAM, accumulate Grams.
                # Lagged one block (process block k-1 here, block NBLK-1
                # after the loop) so the gram matmuls never stall PE on the
                # merge->DMA->transpose chain of the current block.
                if k > 0:
                    _gram_block(k - 1)

            _gram_block(NBLK - 1)

            # ---- phase 1.5: per-batch-group AllReduce --------------------
            # prefetch phase-3's first two v chunks behind the collective
            v_pre = []
            for g in range(2):
                grp = []
                for t in range(OT_V):
                    vt_ = vinp.tile([128, 1024], BF, tag=f"vin{t}",
                                    name=f"vpre{g}_{t}")
                    nc.scalar.dma_start(
                        vt_[:], v_dram[t][:, g * 1024 : (g + 1) * 1024]
                    )
                    grp.append(vt_)
                v_pre.append(grp)
            ccin_sb = smallp.tile([96, 768], F32, tag="ccin")
            for g in range(2):
                nc.scalar.copy(
                    ccin_sb[:, g * 384 : (g + 1) * 384], gram_ps[g][:]
                )
            nc.sync.dma_start(cc_in[:], ccin_sb[:])
            nc.gpsimd.collective_compute(
                "AllReduce",
                AOP.add,
                replica_groups=[[0, 1, 2, 3], [4, 5, 6, 7]],
                ins=[cc_in.opt()],
                outs=[cc_out.opt()],
            )
            gred = smallp.tile([96, 768], F32, tag="gred")
            nc.sync.dma_start(gred[:], cc_out[:])

            # ---- phase 2: norms, scaling, softmax ------------------------
            # diag -> per-channel sum of squares [96(joint c), 8(head)]
            dm = smallp.tile([96, 768], F32, tag="dm")
            nc.vector.tensor_tensor(dm[:], gred[:], eyem_sb[:], AOP.mult)
            dsum = smallp.tile([96, HEADS], F32, tag="dsum")
            nc.vector.tensor_reduce(
                dsum[:],
                dm[:].rearrange("p (h d) -> p h d", d=96),
                axis=mybir.AxisListType.X,
                op=AOP.add,
            )
            norms = smallp.tile([96, HEADS], F32, tag="norms")
            nc.scalar.sqrt(norms[:], dsum[:])
            nc.vector.tensor_scalar_max(norms[:], norms[:], 1e-12)
            rsc = smallp.tile([96, HEADS], F32, tag="rsc")
            nc.vector.reciprocal(rsc[:], norms[:])

            # rk broadcast [48, h*48+d] = rsc[48+d, h]:
            # transpose rsc -> rscT [8, 96]; rkrep[h', (h,d)] = rscT[h', 48+d]
            # masked by delta(h'=h); then ones[8,48].T @ rkrep sums out h'.
            rscb = smallp.tile([96, HEADS], BF, tag="rscb")
            nc.vector.tensor_copy(rscb[:], rsc[:])
            rscT_ps = psC.tile([128, 128], BF, tag="conv")
            nc.tensor.transpose(
                rscT_ps[:HEADS, :96], rscb[:], eyeb_sb[:96, :96]
            )
            rscT = smallp.tile([HEADS, 96], BF, tag="rscT")
            nc.vector.tensor_copy(rscT[:], rscT_ps[:HEADS, :96])
            rkrep = smallp.tile([HEADS, DIM], BF, tag="rkrep")
            mask3d = mask8_sb[:].rearrange("p (h d) -> p h d", d=HD)
            rk3d = rscT[:, HD : 2 * HD].rearrange("p (o d) -> p o d", o=1)
            mask3d, rk3d = bass.broadcast_tensor_aps(mask3d, rk3d)
            nc.vector.tensor_tensor(
                rkrep[:].rearrange("p (h d) -> p h d", d=HD),
                mask3d,
                rk3d,
                AOP.mult,
            )
            rkb_ps = psA.tile([128, 512], F32, tag="gemm")
            nc.tensor.matmul(
                rkb_ps[:HD, :DIM],
                lhsT=ones_sb[:],
                rhs=rkrep[:],
                start=True,
                stop=True,
            )
            # logits L[c, h, d] = G_qk * rk * (temp_h * rq)
            L = smallp.tile([HD, DIM], F32, tag="L")
            gqk = gred[0:HD].rearrange("p (h d) -> p h d", d=96)[:, :, HD : 2 * HD]
            nc.vector.tensor_tensor(
                L[:].rearrange("p (h d) -> p h d", d=HD),
                gqk,
                rkb_ps[:HD, :DIM].rearrange("p (h d) -> p h d", d=HD),
                AOP.mult,
            )
            tsc = smallp.tile([HD, HEADS], F32, tag="tsc")
            nc.vector.tensor_tensor(tsc[:], tempb_sb[:], rsc[0:HD, :], AOP.mult)
            for h in range(HEADS):
                nc.vector.tensor_scalar_mul(
                    L[:, h * HD : (h + 1) * HD],
                    L[:, h * HD : (h + 1) * HD],
                    tsc[:, h : h + 1],
                )
            # softmax over d (free dim, per 48-block)
            mx = smallp.tile([HD, HEADS], F32, tag="mx")
            nc.vector.tensor_reduce(
                mx[:],
                L[:].rearrange("p (h d) -> p h d", d=HD),
                axis=mybir.AxisListType.X,
                op=AOP.max,
            )
            for h in range(HEADS):
                nc.vector.tensor_scalar_sub(
                    L[:, h * HD : (h + 1) * HD],
                    L[:, h * HD : (h + 1) * HD],
                    mx[:, h : h + 1],
                )
            nc.scalar.activation(L[:], L[:], ACT.Exp)
            sm = smallp.tile([HD, HEADS], F32, tag="sm")
            nc.vector.tensor_reduce(
                sm[:],
                L[:].rearrange("p (h d) -> p h d", d=HD),
                axis=mybir.AxisListType.X,
                op=AOP.add,
            )
            rs = smallp.tile([HD, HEADS], F32, tag="rs")
            nc.vector.reciprocal(rs[:], sm[:])
            for h in range(HEADS):
                nc.vector.tensor_scalar_mul(
                    L[:, h * HD : (h + 1) * HD],
                    L[:, h * HD : (h + 1) * HD],
                    rs[:, h : h + 1],
                )
            Lb = smallp.tile([HD, DIM], BF, tag="Lb")
            nc.vector.tensor_copy(Lb[:], L[:])

            # ---- phase 2.5: fold A into proj: P^T[48h+j, o] =
            # sum_i A_h[i,j] * proj[o, 48h+i]; lhsT = Lb[:, h*48+j] slice,
            # rhs = wprojH[:, h*384 : (h+1)*384]. Each head computed at
            # partition 0 in PSUM, then scalar-copied into the P^T tiles
            # (split where a head straddles a 128-partition boundary).
            PT_sb = []
            for ct in range(CT):
                t = smallp.tile([128, DIM], BF, tag=f"PT{ct}")
                PT_sb.append(t)
            for h in range(HEADS):
                ps_h = psA.tile([128, 512], F32, tag="gemm", name=f"ptps{h}")
                nc.tensor.matmul(
                    ps_h[:HD, :DIM],
                    lhsT=Lb[:, h * HD : (h + 1) * HD],
                    rhs=wprojH_sb[:, h * DIM : (h + 1) * DIM],
                    start=True,
                    stop=True,
                )
                stage = smallp.tile([HD, DIM], BF, tag="ptstage", bufs=2)
                nc.scalar.copy(stage[:], ps_h[:HD, :DIM])
                r0 = h * HD
                ct0 = r0 // 128
                split = (ct0 + 1) * 128
                if r0 + HD <= split:
                    pieces = [(ct0, r0 - ct0 * 128, 0, HD)]
                else:
                    pieces = [
                        (ct0, r0 - ct0 * 128, 0, split - r0),
                        (ct0 + 1, 0, split - r0, r0 + HD - split),
                    ]
                # engines need 32-aligned partition bases; DMA does not.
                for ct, row0, joff, jlen in pieces:
                    nc.sync.dma_start(
                        PT_sb[ct][row0 : row0 + jlen, :],
                        stage[joff : joff + jlen, :],
                    )

            # ---- phase 3: fused (proj @ A) @ v_dw GEMM -------------------
            VCH = 1024  # v reload chunk

            ngrp = NLOC // VCH
            pending = list(v_pre)  # FIFO of in-flight v chunk groups
            for nt in range(NLOC // 512):
                if nt % (VCH // 512) == 0:
                    g = nt // (VCH // 512)
                    v_sb = pending.pop(0)
                    gn = g + 2  # keep 2 groups in flight (0,1 preloaded)
                    if gn < ngrp:
                        grp = []
                        for t in range(OT_V):
                            vt_ = vinp.tile([128, VCH], BF, tag=f"vin{t}",
                                            name=f"vg{gn}_{t}")
                            nc.sync.dma_start(
                                vt_[:],
                                v_dram[t][:, gn * VCH : (gn + 1) * VCH],
                            )
                            grp.append(vt_)
                        pending.append(grp)
                off = (nt % (VCH // 512)) * 512
                for po in range(CT):
                    ps = psA.tile([128, 512], F32, tag="gemm", name=f"y{nt}_{po}")
                    for t in range(OT_V):
                        nc.tensor.matmul(
                            ps[:, :512],
                            lhsT=PT_sb[t][:, po * 128 : (po + 1) * 128],
                            rhs=v_sb[t][:, off : off + 512],
                            start=(t == 0),
                            stop=(t == OT_V - 1),
                        )
                    ysb = youtp.tile([128, 512], F32, tag="ysb", name=f"ys{nt}_{po}", bufs=4)
                    if po % 2 == 0:
                        nc.scalar.copy(ysb[:], ps[:, :512])
                        nc.scalar.dma_start(
                            y[po][:, nt * 512 : (nt + 1) * 512], ysb[:]
                        )
                    else:
                        nc.vector.tensor_copy(ysb[:], ps[:, :512])
                        nc.gpsimd.dma_start(
                            y[po][:, nt * 512 : (nt + 1) * 512], ysb[:]
                        )

    nc.compile()
    return nc


_NC = None


def _get_program():
    global _NC
    if _NC is None:
        _NC = _build_program()
    return _NC


def _prep_weights(qkv_w, dw_w, proj_w, log_temp):
    """Host-side weight permutation/padding. Returns dict of shared inputs."""
    qkv_w = np.asarray(qkv_w, np.float32)
    dw_w = np.asarray(dw_w, np.float32).reshape(3 * DIM, 9)
    proj_w = np.asarray(proj_w, np.float32)
    temp = np.log1p(np.exp(np.asarray(log_temp, np.float32).reshape(HEADS)))
    temp = temp + 1e-6

    # permutation: first 768 = per head [q_h | k_h]; then v in natural order
    perm = np.concatenate(
        [
            np.concatenate([np.arange(h * HD, (h + 1) * HD),
                            DIM + np.arange(h * HD, (h + 1) * HD)])
            for h in range(HEADS)
        ]
        + [2 * DIM + np.arange(DIM)]
    )
    wq = qkv_w[perm].copy()
    wd = dw_w[perm].copy()
    # prescale qk 1x1 weights into fp8's range; undo in the dw taps
    wq[:CQK] *= QKSCALE
    wd[:CQK] /= QKSCALE

    wqkvT = np.ascontiguousarray(wq.T.reshape(CT, 128, COUT)).astype(BF16)
    wdw = np.ascontiguousarray(wd.reshape(OT, 128, 9))

    # fp8 DoubleRow stationary: wq8[k, j, m] = wq[m, 128j + k], m < 768
    wq8 = np.zeros((128, 2, CQK), np.float32)
    for j in range(2):
        wq8[:, j, :] = wq[:CQK, 128 * j : 128 * (j + 1)].T
    wq8 = np.ascontiguousarray(wq8.reshape(128, 2 * CQK)).astype(FP8)


    # wprojH[i, h*384 + o] = proj_w[o, 48h + i]
    wprojH = np.zeros((HD, HEADS * DIM), np.float32)
    for h in range(HEADS):
        wprojH[:, h * DIM : (h + 1) * DIM] = proj_w[:, h * HD : (h + 1) * HD].T
    wprojH = wprojH.astype(BF16)

    tempb = np.broadcast_to(temp[None, :], (HD, HEADS)).copy()
    eyeb = np.eye(128, dtype=np.float32).astype(BF16)
    eyem = np.tile(np.eye(96, dtype=np.float32), (1, 8)).copy()
    mask8 = np.repeat(np.eye(HEADS, dtype=np.float32), HD, axis=1).astype(BF16)
    wdiag = np.zeros((OT, len(WDIAG_TAPS), 128, 128), np.float32)
    for ot in range(OT):
        for i, s in enumerate(WDIAG_TAPS):
            np.fill_diagonal(wdiag[ot, i], wd[ot * 128 : (ot + 1) * 128, s])
    # sbuf layout: [128 part(k), ntaps*128 free(s, m)]
    wdiag = np.ascontiguousarray(wdiag.transpose(0, 2, 1, 3)).reshape(
        OT, 128, len(WDIAG_TAPS) * 128
    ).astype(BF16)
    return {
        "wqkvT": wqkvT,
        "wq8": wq8,
        "wdw": wdw,
        "wprojH": wprojH,
        "tempb": tempb,
        "eyeb": eyeb,
        "eyem": eyem,
        "mask8": mask8,
        "wdiag": wdiag,
    }


def _prep_x(x):
    """Per-core padded x chunks: bf16 [CT, 128, HP*WP] and the fp8
    interleaved copy of c-tiles 0,1 ([128, 2*X8W])."""
    x = np.asarray(x, np.float32)
    chunks = []
    for c in range(N_CORES):
        b, r0 = c // 4, ROWS * (c % 4)
        buf = np.zeros((DIM, HP, WP), np.float32)
        lo, hi = max(r0 - 1, 0), min(r0 + ROWS + 1, HGT)
        buf[:, lo - (r0 - 1) : hi - (r0 - 1), 1 : WID + 1] = x[b, :, lo:hi, :]
        flat = buf.reshape(CT, 128, HP * WP)
        x8 = np.zeros((128, 2, X8W), np.float32)
        x8[:, :, : HP * WP] = flat[:2].transpose(1, 0, 2)
        chunks.append((
            np.ascontiguousarray(flat).astype(BF16),
            np.ascontiguousarray(x8.reshape(128, 2 * X8W)).astype(FP8),
        ))
    return chunks


def _run(x, qkv_w, dw_w, proj_w, log_temp, trace=False):
    nc = _get_program()
    shared = _prep_weights(qkv_w, dw_w, proj_w, log_temp)
    xchunks = _prep_x(x)
    in_maps = [
        {**shared, "xp": xchunks[c][0], "xp8": xchunks[c][1]}
        for c in range(N_CORES)
    ]
    res = run_bass_kernel_spmd(
        nc, in_maps, core_ids=list(range(N_CORES)), trace=trace
    )
    out = np.empty((B, DIM, HGT, WID), np.float32)
    for c in range(N_CORES):
        b, r0 = c // 4, ROWS * (c % 4)
        yc = res.results[c]["y"].reshape(DIM, ROWS, WID)
        out[b, :, r0 : r0 + ROWS, :] = yc
    return out, res


def kernel(x, qkv_w, dw_w, proj_w, log_temp):
    out, _ = _run(x, qkv_w, dw_w, proj_w, log_temp, trace=False)
    return out


# revision 6
# speedup vs baseline: 1.0443x; 1.0022x over previous
"""MDTA (Restormer channel attention) Bass/Tile kernel for 8 Trainium2 cores.

Sharding: spatial. Core c handles batch b=c//4, image rows 64*(c%4) .. +64.
The channel attention Gram G = Q K^T and the L2 norms are sums over spatial
positions, so each core accumulates per-head joint [q|k] 96x96 Gram partials
locally and one tiny (2 x 96 x 768 fp32) AllReduce combines them; the
normalization (F.normalize) is applied afterwards as row/col scaling of G.

v2 changes vs v1:
  - v channels unpadded (384, proj order). COUT 1280 -> 1152 (9 o-tiles).
  - attention A folded into project_out on device: P^T = blockdiag(A)^T-free
    per-head matmuls (lhsT = softmax L slice, rhs = per-head projT), so
    phase 3 is a single GEMM y = P^T.T @ v_dw. No A^T transposes, no
    separate apply pass.
  - depthwise 3x3 split 5 DVE taps / 4 PE diag-matmul taps; scalar engine
    evacuates the PE partial, DVE does the final 2x-mode merge add.
  - q/k 1x1-conv GEMM in fp8 (DoubleRow over c-tiles 0+1, weights
    prescaled by QKSCALE, undone in the dw taps; full-width matmuls only
    -- column-split DoubleRow corrupts PSUM).
  - per-ot GEMM->depthwise interleave on PE so the DVE tap chain + merge
    never stalls on a whole-block GEMM section.
  - gram DMA transposes issued from the (idle) sync engine.

Device channel layout (host pre-permutes all weights to match):
  o-tiles 0..5  (768 ch): per head h: [q_h (48) | k_h (48)] interleaved -> the
                joint per-head Gram block is contiguous, and its diagonal
                gives ssq/ssk for the L2 norms.
  o-tiles 6..8  (384 ch): v in natural (proj input) order.
"""

import sys

for p in ("/opt/trn_rl_repo", "/opt/pypackages"):
    if p not in sys.path:
        sys.path.insert(0, p)

import numpy as np
import ml_dtypes

import concourse.bass as bass
import concourse.mybir as mybir
import concourse.tile as tile
import concourse.bacc as bacc
from concourse.bass_utils import run_bass_kernel_spmd

BF16 = ml_dtypes.bfloat16

B, DIM, HGT, WID = 2, 384, 256, 256
HEADS = 8
HD = DIM // HEADS  # 48
N_CORES = 8
ROWS = HGT // 4  # 64 output rows per core
WP = WID + 2  # 258 padded width
HP = ROWS + 2  # 66 padded rows per core
NBLK = 8  # row super-blocks per core
BR = ROWS // NBLK  # 8 output rows per block
BP = BR + 2  # 10 padded rows per block
BN = BR * WID  # 2048 output cols per block
NLOC = ROWS * WID  # 16384 output cols per core

CQK = 2 * DIM  # 768 interleaved q/k channels
CV = DIM  # 384 v channels (unpadded, proj order)
COUT = CQK + CV  # 1152 total device channels
OT_QK = CQK // 128  # 6
OT_V = CV // 128  # 3
OT = OT_QK + OT_V  # 9
CT = DIM // 128  # 3 x c-tiles
GSTRIDE = WP * BP  # 2580 cols per block GEMM
# GEMM moving chunks: 16B-aligned starts (DoubleRow AP requirement)
GCHUNKS = ((0, 864), (864, 864), (1728, 852))
WDIAG_TAPS = (4, 5, 6, 7, 8)  # diag-matmul slots in wdiag (taps on PE)
# per-ot alternation: even ots run tap 4 on DVE (5 DVE / 4 PE taps),
# odd ots run it on PE (4 DVE / 5 PE taps) -- balances the two engines
X8W = 17040  # HP*WP (17028) padded to %16 for the fp8 interleave stride
X8C = 2592  # per-block fp8 chunk width (2580 padded to %16)

F32 = mybir.dt.float32
BF = mybir.dt.bfloat16
F8 = mybir.dt.float8e4
FP8 = ml_dtypes.float8_e4m3
QKSCALE = 128.0  # qk weight prescale into fp8's good range (undone in dw)


def _build_program():
    nc = bacc.Bacc(
        "TRN2",
        target_bir_lowering=False,
        debug=False,
        num_devices=N_CORES,
    )

    xp = nc.dram_tensor("xp", [CT, 128, HP * WP], BF, kind="ExternalInput")
    xp8 = nc.dram_tensor("xp8", [128, 2 * X8W], F8, kind="ExternalInput")
    wq8 = nc.dram_tensor("wq8", [128, 2 * CQK], F8, kind="ExternalInput")
    wqkvT = nc.dram_tensor("wqkvT", [CT, 128, COUT], BF, kind="ExternalInput")
    wdw = nc.dram_tensor("wdw", [OT, 128, 9], F32, kind="ExternalInput")
    wprojH = nc.dram_tensor("wprojH", [HD, HEADS * DIM], BF, kind="ExternalInput")
    tempb = nc.dram_tensor("tempb", [HD, HEADS], F32, kind="ExternalInput")
    eyeb = nc.dram_tensor("eyeb", [128, 128], BF, kind="ExternalInput")
    eyem = nc.dram_tensor("eyem", [96, 768], F32, kind="ExternalInput")
    mask8 = nc.dram_tensor("mask8", [HEADS, DIM], BF, kind="ExternalInput")
    wdiag = nc.dram_tensor(
        "wdiag", [OT, 128, len(WDIAG_TAPS) * 128], BF, kind="ExternalInput"
    )
    y = nc.dram_tensor("y", [CT, 128, NLOC], F32, kind="ExternalOutput")

    AOP = mybir.AluOpType
    ACT = mybir.ActivationFunctionType

    with tile.TileContext(nc) as tc:
        with (
            tc.tile_pool(name="const", bufs=1) as constp,
            tc.tile_pool(name="xin", bufs=1) as xinp,
            tc.tile_pool(name="pre", bufs=1) as prep,
            tc.tile_pool(name="acc", bufs=1) as accp,
            tc.tile_pool(name="qkc", bufs=1) as qkcp,
            tc.tile_pool(name="vc", bufs=1) as vcp,
            tc.tile_pool(name="qkt", bufs=2) as qktp,
            tc.tile_pool(name="small", bufs=1) as smallp,
            tc.tile_pool(name="vin", bufs=2) as vinp,
            tc.tile_pool(name="yout", bufs=2) as youtp,
            tc.tile_pool(name="psA", bufs=3, space="PSUM") as psA,
            tc.tile_pool(name="psC", bufs=2, space="PSUM") as psC,
            tc.tile_pool(name="psG", bufs=1, space="PSUM") as psG,
            tc.tile_pool(name="dram", bufs=1, space="DRAM") as dramp,
        ):
            # ---- resident constants --------------------------------------
            wq_sb = []
            for ct in range(CT):
                t = constp.tile([128, COUT], BF, tag=f"wq{ct}")
                nc.sync.dma_start(t[:], wqkvT[ct])
                wq_sb.append(t)
            wq8_sb = constp.tile([128, 2 * CQK], F8, tag="wq8")
            nc.sync.dma_start(wq8_sb[:], wq8[:])
            wdw_sb = []
            for ot in range(OT):
                t = constp.tile([128, 9], F32, tag=f"wdw{ot}")
                nc.sync.dma_start(t[:], wdw[ot])
                wdw_sb.append(t)
            wprojH_sb = constp.tile([HD, HEADS * DIM], BF, tag="wprojH")
            nc.sync.dma_start(wprojH_sb[:], wprojH[:])
            tempb_sb = constp.tile([HD, HEADS], F32, tag="tempb")
            nc.sync.dma_start(tempb_sb[:], tempb[:])
            eyeb_sb = constp.tile([128, 128], BF, tag="eyeb")
            nc.sync.dma_start(eyeb_sb[:], eyeb[:])
            eyem_sb = constp.tile([96, 768], F32, tag="eyem")
            nc.sync.dma_start(eyem_sb[:], eyem[:])
            ones_sb = constp.tile([HEADS, HD], BF, tag="ones")
            nc.vector.memset(ones_sb[:], 1.0)
            mask8_sb = constp.tile([HEADS, DIM], BF, tag="mask8")
            nc.sync.dma_start(mask8_sb[:], mask8[:])
            wdiag_sb = []
            for ot in range(OT):
                t = constp.tile([128, len(WDIAG_TAPS) * 128], BF, tag=f"wdiag{ot}")
                nc.sync.dma_start(t[:], wdiag[ot])
                wdiag_sb.append(t)

            v_dram = dramp.tile([OT_V, 128, NLOC], BF)
            qk_dram = dramp.tile([OT_QK, 128, NLOC], BF)
            cc_in = dramp.tile([96, 768], F32)
            cc_out = dramp.tile([96, 768], F32)

            # Gram accumulators: 2 banks x [96, 4*96] (4 heads per bank)
            gram_ps = [
                psG.tile([96, 384], F32, tag=f"g{i}", name=f"gram{i}")
                for i in range(2)
            ]

            # ---- phase 1: stream row blocks ------------------------------
            qkd2 = qk_dram[:, :, :].rearrange("t p n -> (t p) n")

            def _gram_block(kb, halves=None):
                for half in halves if halves is not None else range(BN // 128):
                    nt = kb * (BN // 128) + half
                    qkT = qktp.tile([128, CQK], BF, tag="qkT")
                    nc.sync.dma_start_transpose(
                        qkT[:], qkd2[:, nt * 128 : (nt + 1) * 128]
                    )
                    first = kb == 0 and half == 0
                    last = kb == NBLK - 1 and half == BN // 128 - 1
                    for h in range(HEADS):
                        nc.tensor.matmul(
                            gram_ps[h // 4][:, (h % 4) * 96 : (h % 4) * 96 + 96],
                            lhsT=qkT[:, h * 96 : h * 96 + 96],
                            rhs=qkT[:, h * 96 : h * 96 + 96],
                            start=first,
                            stop=last,
                            skip_group_check=True,
                        )

            for k in range(NBLK):
                # x rows 8k .. 8k+10 (padded indexing), all 3 c-tiles
                x_sb = []
                for ct in range(CT):
                    t = xinp.tile([128, GSTRIDE], BF, tag=f"x{ct}")
                    nc.sync.dma_start(
                        t[:], xp[ct][:, k * BR * WP : k * BR * WP + GSTRIDE]
                    )
                    x_sb.append(t)
                # fp8 interleaved view of c-tiles 0,1 for the DoubleRow GEMM
                x8_sb = xinp.tile([128, 2 * X8C], F8, tag="x8")
                for j in range(2):
                    nc.sync.dma_start(
                        x8_sb[:, j * X8C : j * X8C + GSTRIDE],
                        xp8[:, j * X8W + k * BR * WP : j * X8W + k * BR * WP + GSTRIDE],
                    )
                x8r = x8_sb[:].rearrange("p (j n) -> p j n", j=2)
                wq8r = wq8_sb[:].rearrange("p (j m) -> p j m", j=2)

                # qkv pointwise GEMM. qk o-tiles: one fp8 DoubleRow matmul
                # (c-tiles 0+1, weights prescaled by QKSCALE; undone in the
                # dw taps) + one bf16 matmul (c-tile 2). v o-tiles: bf16.
                # Chunk starts 16B-aligned (DoubleRow moving-AP rule); a
                # DoubleRow matmul must be full-width (no column splits).
                pre_sb = []
                for ot in range(OT):
                    t = prep.tile([128, GSTRIDE], BF, tag=f"pre{ot}",
                                  name=f"pre{k}_{ot}")
                    pre_sb.append(t)
                if k == 0:
                    gchunks = ((0, 432), (432, 432), (864, 432),
                               (1296, 432), (1728, 432), (2160, 420))
                else:
                    # rolling halo: rows 8k-1,8k (cols 0..512) are copied
                    # from the previous block's tail (same SBUF slot, so
                    # tap APs stay contiguous); GEMM recomputes only cols
                    # 512..2580. Chunk starts stay 16-aligned for DoubleRow.
                    gchunks = ((512, 432), (944, 432), (1376, 432),
                               (1808, 432), (2240, 340))
                # per-ot pipeline: GEMM(ot) then depthwise(ot) on PE, so
                # the DVE chain + merge for ot never waits on a whole-block
                # PE GEMM section (kills the phase-1 PE/DVE alternation).
                for ot in range(OT):
                    if k > 0:
                        nc.scalar.copy(
                            pre_sb[ot][:, 0:512],
                            pre_sb[ot][:, 2064:2576],
                        )
                    for g0, glen in gchunks:
                        ps = psA.tile([128, 512], F32, tag="gemm")
                        if ot < OT_QK:
                            nc.tensor.matmul(
                                ps[:, :glen],
                                lhsT=wq8r[:, :, ot * 128 : (ot + 1) * 128],
                                rhs=x8r[:, :, g0 : g0 + glen],
                                start=True,
                                stop=False,
                                perf_mode=mybir.MatmulPerfMode.DoubleRow,
                            )
                            nc.tensor.matmul(
                                ps[:, :glen],
                                lhsT=wq_sb[2][:, ot * 128 : (ot + 1) * 128],
                                rhs=x_sb[2][:, g0 : g0 + glen],
                                start=False,
                                stop=True,
                            )
                        else:
                            for ct in range(CT):
                                nc.tensor.matmul(
                                    ps[:, :glen],
                                    lhsT=wq_sb[ct][:, ot * 128 : (ot + 1) * 128],
                                    rhs=x_sb[ct][:, g0 : g0 + glen],
                                    start=(ct == 0),
                                    stop=(ct == CT - 1),
                                )
                        nc.scalar.copy(
                            pre_sb[ot][:, g0 : g0 + glen], ps[:, :glen]
                        )

                    # depthwise 3x3 for this ot: DVE_TAPS on DVE, PE_TAPS as
                    # diag-matmul PSUM accumulation; scalar evacuates the PE
                    # partial, DVE merges.
                    pre_r = pre_sb[ot][:].rearrange("p (r w) -> p r w", w=WP)
                    if ot < OT_QK:
                        dst = qkcp.tile([128, BN], BF, tag=f"qk{ot}")
                    else:
                        dst = vcp.tile([128, BN], BF, tag=f"v{ot - OT_QK}")
                    accA = accp.tile([128, BN], BF, tag="accA")
                    accB = accp.tile([128, BN], BF, tag="accB")
                    tmp = accp.tile([128, BN], BF, tag="tmp")
                    pp = accp.tile([128, BN], BF, tag="pp", bufs=2)
                    dve_taps = (0, 1, 2, 3, 4) if ot % 2 == 0 else (0, 1, 2, 3)
                    pe_taps = (5, 6, 7, 8) if ot % 2 == 0 else (4, 5, 6, 7, 8)
                    pair = [accA, accB]
                    for i, s in enumerate(dve_taps):
                        dh, dw = s // 3, s % 3
                        srcap = pre_r[:, dh : dh + BR, dw : dw + WID]
                        wcol = wdw_sb[ot][:, s : s + 1]
                        cur, nxt = pair[(i + 1) % 2], pair[i % 2]
                        if i == 0:
                            nc.vector.tensor_scalar_mul(nxt[:], srcap, wcol)
                        else:
                            nc.vector.tensor_scalar_mul(tmp[:], srcap, wcol)
                            nc.vector.tensor_tensor(
                                nxt[:], cur[:], tmp[:], AOP.add
                            )
                    accD = pair[(len(dve_taps) - 1) % 2]
                    for g in range(4):
                        pc = psC.tile([128, 512], F32, tag="conv")
                        for i, s in enumerate(pe_taps):
                            idx = s - 4  # slot in wdiag (taps 4..8)
                            dh, dw = s // 3, s % 3
                            rhs = pre_r[
                                :, dh + 2 * g : dh + 2 * g + 2, dw : dw + WID
                            ]
                            nc.tensor.matmul(
                                pc[:],
                                lhsT=wdiag_sb[ot][:, idx * 128 : (idx + 1) * 128],
                                rhs=rhs,
                                start=(i == 0),
                                stop=(i == len(pe_taps) - 1),
                            )
                        nc.scalar.copy(pp[:, g * 512 : (g + 1) * 512], pc[:])
                    nc.vector.tensor_tensor(dst[:], accD[:], pp[:], AOP.add)
                    if ot >= OT_QK:
                        vt = ot - OT_QK
                        nc.sync.dma_start(
                            v_dram[vt][:, k * BN : (k + 1) * BN], dst[:]
                        )
                    else:
                        nc.sync.dma_start(
                            qk_dram[ot][:, k * BN : (k + 1) * BN], dst[:]
                        )
                    # lagged gram for block k-1, two n-tiles per ot:
                    # spreads the 16 transposes across the block so the
                    # sync queue never head-of-line-blocks the gram matmuls
                    if k > 0 and ot < 8:
                        _gram_block(k - 1, halves=(2 * ot, 2 * ot + 1))

                prev_pre = pre_sb

            _gram_block(NBLK - 1)

            # ---- phase 1.5: per-batch-group AllReduce --------------------
            # prefetch phase-3's first two v chunks behind the collective
            v_pre = []
            for g in range(2):
                grp = []
                for t in range(OT_V):
                    vt_ = vinp.tile([128, 1024], BF, tag=f"vin{t}",
                                    name=f"vpre{g}_{t}")
                    nc.scalar.dma_start(
                        vt_[:], v_dram[t][:, g * 1024 : (g + 1) * 1024]
                    )
                    grp.append(vt_)
                v_pre.append(grp)
            ccin_sb = smallp.tile([96, 768], F32, tag="ccin")
            for g in range(2):
                nc.scalar.copy(
                    ccin_sb[:, g * 384 : (g + 1) * 384], gram_ps[g][:]
                )
            nc.sync.dma_start(cc_in[:], ccin_sb[:])
            nc.gpsimd.collective_compute(
                "AllReduce",
                AOP.add,
                replica_groups=[[0, 1, 2, 3], [4, 5, 6, 7]],
                ins=[cc_in.opt()],
                outs=[cc_out.opt()],
            )
            gred = smallp.tile([96, 768], F32, tag="gred")
            nc.sync.dma_start(gred[:], cc_out[:])

            # ---- phase 2: norms, scaling, softmax ------------------------
            # diag -> per-channel sum of squares [96(joint c), 8(head)]
            dm = smallp.tile([96, 768], F32, tag="dm")
            nc.vector.tensor_tensor(dm[:], gred[:], eyem_sb[:], AOP.mult)
            dsum = smallp.tile([96, HEADS], F32, tag="dsum")
            nc.vector.tensor_reduce(
                dsum[:],
                dm[:].rearrange("p (h d) -> p h d", d=96),
                axis=mybir.AxisListType.X,
                op=AOP.add,
            )
            norms = smallp.tile([96, HEADS], F32, tag="norms")
            nc.scalar.sqrt(norms[:], dsum[:])
            nc.vector.tensor_scalar_max(norms[:], norms[:], 1e-12)
            rsc = smallp.tile([96, HEADS], F32, tag="rsc")
            nc.vector.reciprocal(rsc[:], norms[:])

            # rk broadcast [48, h*48+d] = rsc[48+d, h]:
            # transpose rsc -> rscT [8, 96]; rkrep[h', (h,d)] = rscT[h', 48+d]
            # masked by delta(h'=h); then ones[8,48].T @ rkrep sums out h'.
            rscb = smallp.tile([96, HEADS], BF, tag="rscb")
            nc.vector.tensor_copy(rscb[:], rsc[:])
            rscT_ps = psC.tile([128, 128], BF, tag="conv")
            nc.tensor.transpose(
                rscT_ps[:HEADS, :96], rscb[:], eyeb_sb[:96, :96]
            )
            rscT = smallp.tile([HEADS, 96], BF, tag="rscT")
            nc.vector.tensor_copy(rscT[:], rscT_ps[:HEADS, :96])
            rkrep = smallp.tile([HEADS, DIM], BF, tag="rkrep")
            mask3d = mask8_sb[:].rearrange("p (h d) -> p h d", d=HD)
            rk3d = rscT[:, HD : 2 * HD].rearrange("p (o d) -> p o d", o=1)
            mask3d, rk3d = bass.broadcast_tensor_aps(mask3d, rk3d)
            nc.vector.tensor_tensor(
                rkrep[:].rearrange("p (h d) -> p h d", d=HD),
                mask3d,
                rk3d,
                AOP.mult,
            )
            rkb_ps = psA.tile([128, 512], F32, tag="gemm")
            nc.tensor.matmul(
                rkb_ps[:HD, :DIM],
                lhsT=ones_sb[:],
                rhs=rkrep[:],
                start=True,
                stop=True,
            )
            # logits L[c, h, d] = G_qk * rk * (temp_h * rq)
            L = smallp.tile([HD, DIM], F32, tag="L")
            gqk = gred[0:HD].rearrange("p (h d) -> p h d", d=96)[:, :, HD : 2 * HD]
            nc.vector.tensor_tensor(
                L[:].rearrange("p (h d) -> p h d", d=HD),
                gqk,
                rkb_ps[:HD, :DIM].rearrange("p (h d) -> p h d", d=HD),
                AOP.mult,
            )
            tsc = smallp.tile([HD, HEADS], F32, tag="tsc")
            nc.vector.tensor_tensor(tsc[:], tempb_sb[:], rsc[0:HD, :], AOP.mult)
            for h in range(HEADS):
                nc.vector.tensor_scalar_mul(
                    L[:, h * HD : (h + 1) * HD],
                    L[:, h * HD : (h + 1) * HD],
                    tsc[:, h : h + 1],
                )
            # softmax over d (free dim, per 48-block)
            mx = smallp.tile([HD, HEADS], F32, tag="mx")
            nc.vector.tensor_reduce(
                mx[:],
                L[:].rearrange("p (h d) -> p h d", d=HD),
                axis=mybir.AxisListType.X,
                op=AOP.max,
            )
            for h in range(HEADS):
                nc.vector.tensor_scalar_sub(
                    L[:, h * HD : (h + 1) * HD],
                    L[:, h * HD : (h + 1) * HD],
                    mx[:, h : h + 1],
                )
            nc.scalar.activation(L[:], L[:], ACT.Exp)
            sm = smallp.tile([HD, HEADS], F32, tag="sm")
            nc.vector.tensor_reduce(
                sm[:],
                L[:].rearrange("p (h d) -> p h d", d=HD),
                axis=mybir.AxisListType.X,
                op=AOP.add,
            )
            rs = smallp.tile([HD, HEADS], F32, tag="rs")
            nc.vector.reciprocal(rs[:], sm[:])
            for h in range(HEADS):
                nc.vector.tensor_scalar_mul(
                    L[:, h * HD : (h + 1) * HD],
                    L[:, h * HD : (h + 1) * HD],
                    rs[:, h : h + 1],
                )
            Lb = smallp.tile([HD, DIM], BF, tag="Lb")
            nc.vector.tensor_copy(Lb[:], L[:])

            # ---- phase 2.5: fold A into proj: P^T[48h+j, o] =
            # sum_i A_h[i,j] * proj[o, 48h+i]; lhsT = Lb[:, h*48+j] slice,
            # rhs = wprojH[:, h*384 : (h+1)*384]. Each head computed at
            # partition 0 in PSUM, then scalar-copied into the P^T tiles
            # (split where a head straddles a 128-partition boundary).
            PT_sb = []
            for ct in range(CT):
                t = smallp.tile([128, DIM], BF, tag=f"PT{ct}")
                PT_sb.append(t)
            for h in range(HEADS):
                ps_h = psA.tile([128, 512], F32, tag="gemm", name=f"ptps{h}")
                nc.tensor.matmul(
                    ps_h[:HD, :DIM],
                    lhsT=Lb[:, h * HD : (h + 1) * HD],
                    rhs=wprojH_sb[:, h * DIM : (h + 1) * DIM],
                    start=True,
                    stop=True,
                )
                stage = smallp.tile([HD, DIM], BF, tag="ptstage", bufs=2)
                nc.scalar.copy(stage[:], ps_h[:HD, :DIM])
                r0 = h * HD
                ct0 = r0 // 128
                split = (ct0 + 1) * 128
                if r0 + HD <= split:
                    pieces = [(ct0, r0 - ct0 * 128, 0, HD)]
                else:
                    pieces = [
                        (ct0, r0 - ct0 * 128, 0, split - r0),
                        (ct0 + 1, 0, split - r0, r0 + HD - split),
                    ]
                # engines need 32-aligned partition bases; DMA does not.
                for ct, row0, joff, jlen in pieces:
                    nc.sync.dma_start(
                        PT_sb[ct][row0 : row0 + jlen, :],
                        stage[joff : joff + jlen, :],
                    )

            # ---- phase 3: fused (proj @ A) @ v_dw GEMM -------------------
            VCH = 1024  # v reload chunk

            ngrp = NLOC // VCH
            pending = list(v_pre)  # FIFO of in-flight v chunk groups
            for nt in range(NLOC // 512):
                if nt % (VCH // 512) == 0:
                    g = nt // (VCH // 512)
                    v_sb = pending.pop(0)
                    gn = g + 2  # keep 2 groups in flight (0,1 preloaded)
                    if gn < ngrp:
                        grp = []
                        for t in range(OT_V):
                            vt_ = vinp.tile([128, VCH], BF, tag=f"vin{t}",
                                            name=f"vg{gn}_{t}")
                            nc.sync.dma_start(
                                vt_[:],
                                v_dram[t][:, gn * VCH : (gn + 1) * VCH],
                            )
                            grp.append(vt_)
                        pending.append(grp)
                off = (nt % (VCH // 512)) * 512
                for po in range(CT):
                    ps = psA.tile([128, 512], F32, tag="gemm", name=f"y{nt}_{po}")
                    for t in range(OT_V):
                        nc.tensor.matmul(
                            ps[:, :512],
                            lhsT=PT_sb[t][:, po * 128 : (po + 1) * 128],
                            rhs=v_sb[t][:, off : off + 512],
                            start=(t == 0),
                            stop=(t == OT_V - 1),
                        )
                    ysb = youtp.tile([128, 512], F32, tag="ysb", name=f"ys{nt}_{po}", bufs=4)
                    if po % 2 == 0:
                        nc.scalar.copy(ysb[:], ps[:, :512])
                        nc.scalar.dma_start(
                            y[po][:, nt * 512 : (nt + 1) * 512], ysb[:]
                        )
                    else:
                        nc.vector.tensor_copy(ysb[:], ps[:, :512])
                        nc.gpsimd.dma_start(
                            y[po][:, nt * 512 : (nt + 1) * 512], ysb[:]
                        )

    nc.compile()
    return nc


_NC = None


def _get_program():
    global _NC
    if _NC is None:
        _NC = _build_program()
    return _NC


def _prep_weights(qkv_w, dw_w, proj_w, log_temp):
    """Host-side weight permutation/padding. Returns dict of shared inputs."""
    qkv_w = np.asarray(qkv_w, np.float32)
    dw_w = np.asarray(dw_w, np.float32).reshape(3 * DIM, 9)
    proj_w = np.asarray(proj_w, np.float32)
    temp = np.log1p(np.exp(np.asarray(log_temp, np.float32).reshape(HEADS)))
    temp = temp + 1e-6

    # permutation: first 768 = per head [q_h | k_h]; then v in natural order
    perm = np.concatenate(
        [
            np.concatenate([np.arange(h * HD, (h + 1) * HD),
                            DIM + np.arange(h * HD, (h + 1) * HD)])
            for h in range(HEADS)
        ]
        + [2 * DIM + np.arange(DIM)]
    )
    wq = qkv_w[perm].copy()
    wd = dw_w[perm].copy()
    # prescale qk 1x1 weights into fp8's range; undo in the dw taps
    wq[:CQK] *= QKSCALE
    wd[:CQK] /= QKSCALE

    wqkvT = np.ascontiguousarray(wq.T.reshape(CT, 128, COUT)).astype(BF16)
    wdw = np.ascontiguousarray(wd.reshape(OT, 128, 9))

    # fp8 DoubleRow stationary: wq8[k, j, m] = wq[m, 128j + k], m < 768
    wq8 = np.zeros((128, 2, CQK), np.float32)
    for j in range(2):
        wq8[:, j, :] = wq[:CQK, 128 * j : 128 * (j + 1)].T
    wq8 = np.ascontiguousarray(wq8.reshape(128, 2 * CQK)).astype(FP8)


    # wprojH[i, h*384 + o] = proj_w[o, 48h + i]
    wprojH = np.zeros((HD, HEADS * DIM), np.float32)
    for h in range(HEADS):
        wprojH[:, h * DIM : (h + 1) * DIM] = proj_w[:, h * HD : (h + 1) * HD].T
    wprojH = wprojH.astype(BF16)

    tempb = np.broadcast_to(temp[None, :], (HD, HEADS)).copy()
    eyeb = np.eye(128, dtype=np.float32).astype(BF16)
    eyem = np.tile(np.eye(96, dtype=np.float32), (1, 8)).copy()
    mask8 = np.repeat(np.eye(HEADS, dtype=np.float32), HD, axis=1).astype(BF16)
    wdiag = np.zeros((OT, len(WDIAG_TAPS), 128, 128), np.float32)
    for ot in range(OT):
        for i, s in enumerate(WDIAG_TAPS):
            np.fill_diagonal(wdiag[ot, i], wd[ot * 128 : (ot + 1) * 128, s])
    # sbuf layout: [128 part(k), ntaps*128 free(s, m)]
    wdiag = np.ascontiguousarray(wdiag.transpose(0, 2, 1, 3)).reshape(
        OT, 128, len(WDIAG_TAPS) * 128
    ).astype(BF16)
    return {
        "wqkvT": wqkvT,
        "wq8": wq8,
        "wdw": wdw,
        "wprojH": wprojH,
        "tempb": tempb,
        "eyeb": eyeb,
        "eyem": eyem,
        "mask8": mask8,
        "wdiag": wdiag,
    }


def _prep_x(x):
    """Per-core padded x chunks: bf16 [CT, 128, HP*WP] and the fp8
    interleaved copy of c-tiles 0,1 ([128, 2*X8W])."""
    x = np.asarray(x, np.float32)
    chunks = []
    for c in range(N_CORES):
        b, r0 = c // 4, ROWS * (c % 4)
        buf = np.zeros((DIM, HP, WP), np.float32)
        lo, hi = max(r0 - 1, 0), min(r0 + ROWS + 1, HGT)
        buf[:, lo - (r0 - 1) : hi - (r0 - 1), 1 : WID + 1] = x[b, :, lo:hi, :]
        flat = buf.reshape(CT, 128, HP * WP)
        x8 = np.zeros((128, 2, X8W), np.float32)
        x8[:, :, : HP * WP] = flat[:2].transpose(1, 0, 2)
        chunks.append((
            np.ascontiguousarray(flat).astype(BF16),
            np.ascontiguousarray(x8.reshape(128, 2 * X8W)).astype(FP8),
        ))
    return chunks


def _run(x, qkv_w, dw_w, proj_w, log_temp, trace=False):
    nc = _get_program()
    shared = _prep_weights(qkv_w, dw_w, proj_w, log_temp)
    xchunks = _prep_x(x)
    in_maps = [
        {**shared, "xp": xchunks[c][0], "xp8": xchunks[c][1]}
        for c in range(N_CORES)
    ]
    res = run_bass_kernel_spmd(
        nc, in_maps, core_ids=list(range(N_CORES)), trace=trace
    )
    out = np.empty((B, DIM, HGT, WID), np.float32)
    for c in range(N_CORES):
        b, r0 = c // 4, ROWS * (c % 4)
        yc = res.results[c]["y"].reshape(DIM, ROWS, WID)
        out[b, :, r0 : r0 + ROWS, :] = yc
    return out, res


def kernel(x, qkv_w, dw_w, proj_w, log_temp):
    out, _ = _run(x, qkv_w, dw_w, proj_w, log_temp, trace=False)
    return out


# revision 7
# speedup vs baseline: 1.0502x; 1.0056x over previous
"""MDTA (Restormer channel attention) Bass/Tile kernel for 8 Trainium2 cores.

Sharding: spatial. Core c handles batch b=c//4, image rows 64*(c%4) .. +64.
The channel attention Gram G = Q K^T and the L2 norms are sums over spatial
positions, so each core accumulates per-head joint [q|k] 96x96 Gram partials
locally and one tiny (2 x 96 x 768 fp32) AllReduce combines them; the
normalization (F.normalize) is applied afterwards as row/col scaling of G.

v2 changes vs v1:
  - v channels unpadded (384, proj order). COUT 1280 -> 1152 (9 o-tiles).
  - attention A folded into project_out on device: P^T = blockdiag(A)^T-free
    per-head matmuls (lhsT = softmax L slice, rhs = per-head projT), so
    phase 3 is a single GEMM y = P^T.T @ v_dw. No A^T transposes, no
    separate apply pass.
  - depthwise 3x3 split 5 DVE taps / 4 PE diag-matmul taps; scalar engine
    evacuates the PE partial, DVE does the final 2x-mode merge add.
  - q/k 1x1-conv GEMM in fp8 (DoubleRow over c-tiles 0+1, weights
    prescaled by QKSCALE, undone in the dw taps; full-width matmuls only
    -- column-split DoubleRow corrupts PSUM).
  - per-ot GEMM->depthwise interleave on PE so the DVE tap chain + merge
    never stalls on a whole-block GEMM section.
  - gram DMA transposes issued from the (idle) sync engine.

Device channel layout (host pre-permutes all weights to match):
  o-tiles 0..5  (768 ch): per head h: [q_h (48) | k_h (48)] interleaved -> the
                joint per-head Gram block is contiguous, and its diagonal
                gives ssq/ssk for the L2 norms.
  o-tiles 6..8  (384 ch): v in natural (proj input) order.
"""

import sys

for p in ("/opt/trn_rl_repo", "/opt/pypackages"):
    if p not in sys.path:
        sys.path.insert(0, p)

import numpy as np
import ml_dtypes

import concourse.bass as bass
import concourse.mybir as mybir
import concourse.tile as tile
import concourse.bacc as bacc
from concourse.bass_utils import run_bass_kernel_spmd

BF16 = ml_dtypes.bfloat16

B, DIM, HGT, WID = 2, 384, 256, 256
HEADS = 8
HD = DIM // HEADS  # 48
N_CORES = 8
ROWS = HGT // 4  # 64 output rows per core
WP = WID + 2  # 258 padded width
HP = ROWS + 2  # 66 padded rows per core
NBLK = 8  # row super-blocks per core
BR = ROWS // NBLK  # 8 output rows per block
BP = BR + 2  # 10 padded rows per block
BN = BR * WID  # 2048 output cols per block
NLOC = ROWS * WID  # 16384 output cols per core

CQK = 2 * DIM  # 768 interleaved q/k channels
CV = DIM  # 384 v channels (unpadded, proj order)
COUT = CQK + CV  # 1152 total device channels
OT_QK = CQK // 128  # 6
OT_V = CV // 128  # 3
OT = OT_QK + OT_V  # 9
CT = DIM // 128  # 3 x c-tiles
GSTRIDE = WP * BP  # 2580 cols per block GEMM
# GEMM moving chunks: 16B-aligned starts (DoubleRow AP requirement)
GCHUNKS = ((0, 864), (864, 864), (1728, 852))
WDIAG_TAPS = (4, 5, 6, 7, 8)  # diag-matmul slots in wdiag (taps on PE)
# per-ot alternation: even ots run tap 4 on DVE (5 DVE / 4 PE taps),
# odd ots run it on PE (4 DVE / 5 PE taps) -- balances the two engines
X8W = 17040  # HP*WP (17028) padded to %16 for the fp8 interleave stride
X8C = 2592  # per-block fp8 chunk width (2580 padded to %16)

F32 = mybir.dt.float32
BF = mybir.dt.bfloat16
F8 = mybir.dt.float8e4
FP8 = ml_dtypes.float8_e4m3
QKSCALE = 128.0  # qk weight prescale into fp8's good range (undone in dw)


def _build_program():
    nc = bacc.Bacc(
        "TRN2",
        target_bir_lowering=False,
        debug=False,
        num_devices=N_CORES,
    )

    xp = nc.dram_tensor("xp", [CT, 128, HP * WP], BF, kind="ExternalInput")
    xp8 = nc.dram_tensor("xp8", [128, 2 * X8W], F8, kind="ExternalInput")
    wq8 = nc.dram_tensor("wq8", [128, 2 * CQK], F8, kind="ExternalInput")
    wqkvT = nc.dram_tensor("wqkvT", [CT, 128, COUT], BF, kind="ExternalInput")
    wdw = nc.dram_tensor("wdw", [OT, 128, 9], F32, kind="ExternalInput")
    wprojH = nc.dram_tensor("wprojH", [HD, HEADS * DIM], BF, kind="ExternalInput")
    tempb = nc.dram_tensor("tempb", [HD, HEADS], F32, kind="ExternalInput")
    eyeb = nc.dram_tensor("eyeb", [128, 128], BF, kind="ExternalInput")
    eyem = nc.dram_tensor("eyem", [96, 768], F32, kind="ExternalInput")
    mask8 = nc.dram_tensor("mask8", [HEADS, DIM], BF, kind="ExternalInput")
    wdiag = nc.dram_tensor(
        "wdiag", [OT, 128, len(WDIAG_TAPS) * 128], BF, kind="ExternalInput"
    )
    y = nc.dram_tensor("y", [CT, 128, NLOC], F32, kind="ExternalOutput")

    AOP = mybir.AluOpType
    ACT = mybir.ActivationFunctionType

    with tile.TileContext(nc) as tc:
        with (
            tc.tile_pool(name="const", bufs=1) as constp,
            tc.tile_pool(name="xin", bufs=1) as xinp,
            tc.tile_pool(name="pre", bufs=1) as prep,
            tc.tile_pool(name="acc", bufs=1) as accp,
            tc.tile_pool(name="qkc", bufs=1) as qkcp,
            tc.tile_pool(name="vc", bufs=1) as vcp,
            tc.tile_pool(name="qkt", bufs=2) as qktp,
            tc.tile_pool(name="small", bufs=1) as smallp,
            tc.tile_pool(name="vin", bufs=2) as vinp,
            tc.tile_pool(name="yout", bufs=2) as youtp,
            tc.tile_pool(name="psA", bufs=3, space="PSUM") as psA,
            tc.tile_pool(name="psC", bufs=2, space="PSUM") as psC,
            tc.tile_pool(name="psG", bufs=1, space="PSUM") as psG,
            tc.tile_pool(name="dram", bufs=1, space="DRAM") as dramp,
        ):
            # ---- resident constants --------------------------------------
            wq_sb = []
            for ct in range(CT):
                t = constp.tile([128, COUT], BF, tag=f"wq{ct}")
                nc.sync.dma_start(t[:], wqkvT[ct])
                wq_sb.append(t)
            wq8_sb = constp.tile([128, 2 * CQK], F8, tag="wq8")
            nc.sync.dma_start(wq8_sb[:], wq8[:])
            wdw_sb = []
            for ot in range(OT):
                t = constp.tile([128, 9], F32, tag=f"wdw{ot}")
                nc.sync.dma_start(t[:], wdw[ot])
                wdw_sb.append(t)
            wprojH_sb = constp.tile([HD, HEADS * DIM], BF, tag="wprojH")
            nc.sync.dma_start(wprojH_sb[:], wprojH[:])
            tempb_sb = constp.tile([HD, HEADS], F32, tag="tempb")
            nc.sync.dma_start(tempb_sb[:], tempb[:])
            eyeb_sb = constp.tile([128, 128], BF, tag="eyeb")
            nc.sync.dma_start(eyeb_sb[:], eyeb[:])
            eyem_sb = constp.tile([96, 768], F32, tag="eyem")
            nc.sync.dma_start(eyem_sb[:], eyem[:])
            ones_sb = constp.tile([HEADS, HD], BF, tag="ones")
            nc.vector.memset(ones_sb[:], 1.0)
            mask8_sb = constp.tile([HEADS, DIM], BF, tag="mask8")
            nc.sync.dma_start(mask8_sb[:], mask8[:])
            wdiag_sb = []
            for ot in range(OT):
                t = constp.tile([128, len(WDIAG_TAPS) * 128], BF, tag=f"wdiag{ot}")
                nc.sync.dma_start(t[:], wdiag[ot])
                wdiag_sb.append(t)

            v_dram = dramp.tile([OT_V, 128, NLOC], BF)
            qk_dram = dramp.tile([OT_QK, 128, NLOC], BF)
            cc_in = dramp.tile([96, 768], F32)
            cc_out = dramp.tile([96, 768], F32)

            # Gram accumulators: 2 banks x [96, 4*96] (4 heads per bank)
            gram_ps = [
                psG.tile([96, 384], F32, tag=f"g{i}", name=f"gram{i}")
                for i in range(2)
            ]

            # ---- phase 1: stream row blocks ------------------------------
            qkd2 = qk_dram[:, :, :].rearrange("t p n -> (t p) n")

            def _gram_block(kb, halves=None):
                for half in halves if halves is not None else range(BN // 128):
                    nt = kb * (BN // 128) + half
                    qkT = qktp.tile([128, CQK], BF, tag="qkT")
                    nc.sync.dma_start_transpose(
                        qkT[:], qkd2[:, nt * 128 : (nt + 1) * 128]
                    )
                    first = kb == 0 and half == 0
                    last = kb == NBLK - 1 and half == BN // 128 - 1
                    for h in range(HEADS):
                        nc.tensor.matmul(
                            gram_ps[h // 4][:, (h % 4) * 96 : (h % 4) * 96 + 96],
                            lhsT=qkT[:, h * 96 : h * 96 + 96],
                            rhs=qkT[:, h * 96 : h * 96 + 96],
                            start=first,
                            stop=last,
                            skip_group_check=True,
                        )

            for k in range(NBLK):
                # x rows 8k .. 8k+10 (padded indexing), all 3 c-tiles
                x_sb = []
                for ct in range(CT):
                    t = xinp.tile([128, GSTRIDE], BF, tag=f"x{ct}", bufs=2, name=f"x{ct}")
                    nc.sync.dma_start(
                        t[:], xp[ct][:, k * BR * WP : k * BR * WP + GSTRIDE]
                    )
                    x_sb.append(t)
                # fp8 interleaved view of c-tiles 0,1 for the DoubleRow GEMM
                x8_sb = xinp.tile([128, 2 * X8C], F8, tag="x8")
                for j in range(2):
                    nc.sync.dma_start(
                        x8_sb[:, j * X8C : j * X8C + GSTRIDE],
                        xp8[:, j * X8W + k * BR * WP : j * X8W + k * BR * WP + GSTRIDE],
                    )
                x8r = x8_sb[:].rearrange("p (j n) -> p j n", j=2)
                wq8r = wq8_sb[:].rearrange("p (j m) -> p j m", j=2)

                # qkv pointwise GEMM. qk o-tiles: one fp8 DoubleRow matmul
                # (c-tiles 0+1, weights prescaled by QKSCALE; undone in the
                # dw taps) + one bf16 matmul (c-tile 2). v o-tiles: bf16.
                # Chunk starts 16B-aligned (DoubleRow moving-AP rule); a
                # DoubleRow matmul must be full-width (no column splits).
                pre_sb = []
                for ot in range(OT):
                    t = prep.tile([128, GSTRIDE], BF, tag=f"pre{ot}",
                                  name=f"pre{k}_{ot}")
                    pre_sb.append(t)
                if k == 0:
                    gchunks = ((0, 432), (432, 432), (864, 432),
                               (1296, 432), (1728, 432), (2160, 420))
                else:
                    # rolling halo: rows 8k-1,8k (cols 0..512) are copied
                    # from the previous block's tail (same SBUF slot, so
                    # tap APs stay contiguous); GEMM recomputes only cols
                    # 512..2580. Chunk starts stay 16-aligned for DoubleRow.
                    gchunks = ((512, 432), (944, 432), (1376, 432),
                               (1808, 432), (2240, 340))
                # per-ot pipeline: GEMM(ot) then depthwise(ot) on PE, so
                # the DVE chain + merge for ot never waits on a whole-block
                # PE GEMM section (kills the phase-1 PE/DVE alternation).
                for ot in range(OT):
                    if k > 0:
                        nc.scalar.copy(
                            pre_sb[ot][:, 0:512],
                            pre_sb[ot][:, 2064:2576],
                        )
                    for g0, glen in gchunks:
                        ps = psA.tile([128, 512], F32, tag="gemm")
                        if ot < OT_QK:
                            nc.tensor.matmul(
                                ps[:, :glen],
                                lhsT=wq8r[:, :, ot * 128 : (ot + 1) * 128],
                                rhs=x8r[:, :, g0 : g0 + glen],
                                start=True,
                                stop=False,
                                perf_mode=mybir.MatmulPerfMode.DoubleRow,
                            )
                            nc.tensor.matmul(
                                ps[:, :glen],
                                lhsT=wq_sb[2][:, ot * 128 : (ot + 1) * 128],
                                rhs=x_sb[2][:, g0 : g0 + glen],
                                start=False,
                                stop=True,
                            )
                        else:
                            for ct in range(CT):
                                nc.tensor.matmul(
                                    ps[:, :glen],
                                    lhsT=wq_sb[ct][:, ot * 128 : (ot + 1) * 128],
                                    rhs=x_sb[ct][:, g0 : g0 + glen],
                                    start=(ct == 0),
                                    stop=(ct == CT - 1),
                                )
                        nc.scalar.copy(
                            pre_sb[ot][:, g0 : g0 + glen], ps[:, :glen]
                        )

                    # depthwise 3x3 for this ot: DVE_TAPS on DVE, PE_TAPS as
                    # diag-matmul PSUM accumulation; scalar evacuates the PE
                    # partial, DVE merges.
                    pre_r = pre_sb[ot][:].rearrange("p (r w) -> p r w", w=WP)
                    if ot < OT_QK:
                        dst = qkcp.tile([128, BN], BF, tag=f"qk{ot}")
                    else:
                        dst = vcp.tile([128, BN], BF, tag=f"v{ot - OT_QK}")
                    accA = accp.tile([128, BN], BF, tag="accA")
                    accB = accp.tile([128, BN], BF, tag="accB")
                    tmp = accp.tile([128, BN], BF, tag="tmp")
                    pp = accp.tile([128, BN], BF, tag="pp", bufs=2)
                    dve_taps = (0, 1, 2, 3, 4) if ot % 2 == 0 else (0, 1, 2, 3)
                    pe_taps = (5, 6, 7, 8) if ot % 2 == 0 else (4, 5, 6, 7, 8)
                    pair = [accA, accB]
                    for i, s in enumerate(dve_taps):
                        dh, dw = s // 3, s % 3
                        srcap = pre_r[:, dh : dh + BR, dw : dw + WID]
                        wcol = wdw_sb[ot][:, s : s + 1]
                        cur, nxt = pair[(i + 1) % 2], pair[i % 2]
                        if i == 0:
                            nc.vector.tensor_scalar_mul(nxt[:], srcap, wcol)
                        else:
                            nc.vector.tensor_scalar_mul(tmp[:], srcap, wcol)
                            nc.vector.tensor_tensor(
                                nxt[:], cur[:], tmp[:], AOP.add
                            )
                    accD = pair[(len(dve_taps) - 1) % 2]
                    for g in range(4):
                        pc = psC.tile([128, 512], F32, tag="conv")
                        for i, s in enumerate(pe_taps):
                            idx = s - 4  # slot in wdiag (taps 4..8)
                            dh, dw = s // 3, s % 3
                            rhs = pre_r[
                                :, dh + 2 * g : dh + 2 * g + 2, dw : dw + WID
                            ]
                            nc.tensor.matmul(
                                pc[:],
                                lhsT=wdiag_sb[ot][:, idx * 128 : (idx + 1) * 128],
                                rhs=rhs,
                                start=(i == 0),
                                stop=(i == len(pe_taps) - 1),
                            )
                        nc.scalar.copy(pp[:, g * 512 : (g + 1) * 512], pc[:])
                    nc.vector.tensor_tensor(dst[:], accD[:], pp[:], AOP.add)
                    if ot >= OT_QK:
                        vt = ot - OT_QK
                        nc.sync.dma_start(
                            v_dram[vt][:, k * BN : (k + 1) * BN], dst[:]
                        )
                    else:
                        nc.sync.dma_start(
                            qk_dram[ot][:, k * BN : (k + 1) * BN], dst[:]
                        )
                    # lagged gram for block k-1, two n-tiles per ot:
                    # spreads the 16 transposes across the block so the
                    # sync queue never head-of-line-blocks the gram matmuls
                    if k > 0 and ot < 8:
                        _gram_block(k - 1, halves=(2 * ot, 2 * ot + 1))

                prev_pre = pre_sb

            _gram_block(NBLK - 1)

            # ---- phase 1.5: per-batch-group AllReduce --------------------
            # prefetch phase-3's first two v chunks behind the collective
            v_pre = []
            for g in range(2):
                grp = []
                for t in range(OT_V):
                    vt_ = vinp.tile([128, 1024], BF, tag=f"vin{t}",
                                    name=f"vpre{g}_{t}")
                    nc.scalar.dma_start(
                        vt_[:], v_dram[t][:, g * 1024 : (g + 1) * 1024]
                    )
                    grp.append(vt_)
                v_pre.append(grp)
            ccin_sb = smallp.tile([96, 768], F32, tag="ccin")
            for g in range(2):
                nc.scalar.copy(
                    ccin_sb[:, g * 384 : (g + 1) * 384], gram_ps[g][:]
                )
            nc.sync.dma_start(cc_in[:], ccin_sb[:])
            nc.gpsimd.collective_compute(
                "AllReduce",
                AOP.add,
                replica_groups=[[0, 1, 2, 3], [4, 5, 6, 7]],
                ins=[cc_in.opt()],
                outs=[cc_out.opt()],
            )
            gred = smallp.tile([96, 768], F32, tag="gred")
            nc.sync.dma_start(gred[:], cc_out[:])

            # ---- phase 2: norms, scaling, softmax ------------------------
            # diag -> per-channel sum of squares [96(joint c), 8(head)]
            dm = smallp.tile([96, 768], F32, tag="dm")
            nc.vector.tensor_tensor(dm[:], gred[:], eyem_sb[:], AOP.mult)
            dsum = smallp.tile([96, HEADS], F32, tag="dsum")
            nc.vector.tensor_reduce(
                dsum[:],
                dm[:].rearrange("p (h d) -> p h d", d=96),
                axis=mybir.AxisListType.X,
                op=AOP.add,
            )
            norms = smallp.tile([96, HEADS], F32, tag="norms")
            nc.scalar.sqrt(norms[:], dsum[:])
            nc.vector.tensor_scalar_max(norms[:], norms[:], 1e-12)
            rsc = smallp.tile([96, HEADS], F32, tag="rsc")
            nc.vector.reciprocal(rsc[:], norms[:])

            # rk broadcast [48, h*48+d] = rsc[48+d, h]:
            # transpose rsc -> rscT [8, 96]; rkrep[h', (h,d)] = rscT[h', 48+d]
            # masked by delta(h'=h); then ones[8,48].T @ rkrep sums out h'.
            rscb = smallp.tile([96, HEADS], BF, tag="rscb")
            nc.vector.tensor_copy(rscb[:], rsc[:])
            rscT_ps = psC.tile([128, 128], BF, tag="conv")
            nc.tensor.transpose(
                rscT_ps[:HEADS, :96], rscb[:], eyeb_sb[:96, :96]
            )
            rscT = smallp.tile([HEADS, 96], BF, tag="rscT")
            nc.vector.tensor_copy(rscT[:], rscT_ps[:HEADS, :96])
            rkrep = smallp.tile([HEADS, DIM], BF, tag="rkrep")
            mask3d = mask8_sb[:].rearrange("p (h d) -> p h d", d=HD)
            rk3d = rscT[:, HD : 2 * HD].rearrange("p (o d) -> p o d", o=1)
            mask3d, rk3d = bass.broadcast_tensor_aps(mask3d, rk3d)
            nc.vector.tensor_tensor(
                rkrep[:].rearrange("p (h d) -> p h d", d=HD),
                mask3d,
                rk3d,
                AOP.mult,
            )
            rkb_ps = psA.tile([128, 512], F32, tag="gemm")
            nc.tensor.matmul(
                rkb_ps[:HD, :DIM],
                lhsT=ones_sb[:],
                rhs=rkrep[:],
                start=True,
                stop=True,
            )
            # logits L[c, h, d] = G_qk * rk * (temp_h * rq)
            L = smallp.tile([HD, DIM], F32, tag="L")
            gqk = gred[0:HD].rearrange("p (h d) -> p h d", d=96)[:, :, HD : 2 * HD]
            nc.vector.tensor_tensor(
                L[:].rearrange("p (h d) -> p h d", d=HD),
                gqk,
                rkb_ps[:HD, :DIM].rearrange("p (h d) -> p h d", d=HD),
                AOP.mult,
            )
            tsc = smallp.tile([HD, HEADS], F32, tag="tsc")
            nc.vector.tensor_tensor(tsc[:], tempb_sb[:], rsc[0:HD, :], AOP.mult)
            for h in range(HEADS):
                nc.vector.tensor_scalar_mul(
                    L[:, h * HD : (h + 1) * HD],
                    L[:, h * HD : (h + 1) * HD],
                    tsc[:, h : h + 1],
                )
            # softmax over d (free dim, per 48-block)
            mx = smallp.tile([HD, HEADS], F32, tag="mx")
            nc.vector.tensor_reduce(
                mx[:],
                L[:].rearrange("p (h d) -> p h d", d=HD),
                axis=mybir.AxisListType.X,
                op=AOP.max,
            )
            for h in range(HEADS):
                nc.vector.tensor_scalar_sub(
                    L[:, h * HD : (h + 1) * HD],
                    L[:, h * HD : (h + 1) * HD],
                    mx[:, h : h + 1],
                )
            nc.scalar.activation(L[:], L[:], ACT.Exp)
            sm = smallp.tile([HD, HEADS], F32, tag="sm")
            nc.vector.tensor_reduce(
                sm[:],
                L[:].rearrange("p (h d) -> p h d", d=HD),
                axis=mybir.AxisListType.X,
                op=AOP.add,
            )
            rs = smallp.tile([HD, HEADS], F32, tag="rs")
            nc.vector.reciprocal(rs[:], sm[:])
            for h in range(HEADS):
                nc.vector.tensor_scalar_mul(
                    L[:, h * HD : (h + 1) * HD],
                    L[:, h * HD : (h + 1) * HD],
                    rs[:, h : h + 1],
                )
            Lb = smallp.tile([HD, DIM], BF, tag="Lb")
            nc.vector.tensor_copy(Lb[:], L[:])

            # ---- phase 2.5: fold A into proj: P^T[48h+j, o] =
            # sum_i A_h[i,j] * proj[o, 48h+i]; lhsT = Lb[:, h*48+j] slice,
            # rhs = wprojH[:, h*384 : (h+1)*384]. Each head computed at
            # partition 0 in PSUM, then scalar-copied into the P^T tiles
            # (split where a head straddles a 128-partition boundary).
            PT_sb = []
            for ct in range(CT):
                t = smallp.tile([128, DIM], BF, tag=f"PT{ct}")
                PT_sb.append(t)
            for h in range(HEADS):
                ps_h = psA.tile([128, 512], F32, tag="gemm", name=f"ptps{h}")
                nc.tensor.matmul(
                    ps_h[:HD, :DIM],
                    lhsT=Lb[:, h * HD : (h + 1) * HD],
                    rhs=wprojH_sb[:, h * DIM : (h + 1) * DIM],
                    start=True,
                    stop=True,
                )
                stage = smallp.tile([HD, DIM], BF, tag="ptstage", bufs=2)
                nc.scalar.copy(stage[:], ps_h[:HD, :DIM])
                r0 = h * HD
                ct0 = r0 // 128
                split = (ct0 + 1) * 128
                if r0 + HD <= split:
                    pieces = [(ct0, r0 - ct0 * 128, 0, HD)]
                else:
                    pieces = [
                        (ct0, r0 - ct0 * 128, 0, split - r0),
                        (ct0 + 1, 0, split - r0, r0 + HD - split),
                    ]
                # engines need 32-aligned partition bases; DMA does not.
                for ct, row0, joff, jlen in pieces:
                    nc.sync.dma_start(
                        PT_sb[ct][row0 : row0 + jlen, :],
                        stage[joff : joff + jlen, :],
                    )

            # ---- phase 3: fused (proj @ A) @ v_dw GEMM -------------------
            VCH = 1024  # v reload chunk

            ngrp = NLOC // VCH
            pending = list(v_pre)  # FIFO of in-flight v chunk groups
            for nt in range(NLOC // 512):
                if nt % (VCH // 512) == 0:
                    g = nt // (VCH // 512)
                    v_sb = pending.pop(0)
                    gn = g + 2  # keep 2 groups in flight (0,1 preloaded)
                    if gn < ngrp:
                        grp = []
                        for t in range(OT_V):
                            vt_ = vinp.tile([128, VCH], BF, tag=f"vin{t}",
                                            name=f"vg{gn}_{t}")
                            nc.sync.dma_start(
                                vt_[:],
                                v_dram[t][:, gn * VCH : (gn + 1) * VCH],
                            )
                            grp.append(vt_)
                        pending.append(grp)
                off = (nt % (VCH // 512)) * 512
                for po in range(CT):
                    ps = psA.tile([128, 512], F32, tag="gemm", name=f"y{nt}_{po}")
                    for t in range(OT_V):
                        nc.tensor.matmul(
                            ps[:, :512],
                            lhsT=PT_sb[t][:, po * 128 : (po + 1) * 128],
                            rhs=v_sb[t][:, off : off + 512],
                            start=(t == 0),
                            stop=(t == OT_V - 1),
                        )
                    ysb = youtp.tile([128, 512], F32, tag="ysb", name=f"ys{nt}_{po}", bufs=4)
                    if po % 2 == 0:
                        nc.scalar.copy(ysb[:], ps[:, :512])
                        nc.scalar.dma_start(
                            y[po][:, nt * 512 : (nt + 1) * 512], ysb[:]
                        )
                    else:
                        nc.vector.tensor_copy(ysb[:], ps[:, :512])
                        nc.gpsimd.dma_start(
                            y[po][:, nt * 512 : (nt + 1) * 512], ysb[:]
                        )

    nc.compile()
    return nc


_NC = None


def _get_program():
    global _NC
    if _NC is None:
        _NC = _build_program()
    return _NC


def _prep_weights(qkv_w, dw_w, proj_w, log_temp):
    """Host-side weight permutation/padding. Returns dict of shared inputs."""
    qkv_w = np.asarray(qkv_w, np.float32)
    dw_w = np.asarray(dw_w, np.float32).reshape(3 * DIM, 9)
    proj_w = np.asarray(proj_w, np.float32)
    temp = np.log1p(np.exp(np.asarray(log_temp, np.float32).reshape(HEADS)))
    temp = temp + 1e-6

    # permutation: first 768 = per head [q_h | k_h]; then v in natural order
    perm = np.concatenate(
        [
            np.concatenate([np.arange(h * HD, (h + 1) * HD),
                            DIM + np.arange(h * HD, (h + 1) * HD)])
            for h in range(HEADS)
        ]
        + [2 * DIM + np.arange(DIM)]
    )
    wq = qkv_w[perm].copy()
    wd = dw_w[perm].copy()
    # prescale qk 1x1 weights into fp8's range; undo in the dw taps
    wq[:CQK] *= QKSCALE
    wd[:CQK] /= QKSCALE

    wqkvT = np.ascontiguousarray(wq.T.reshape(CT, 128, COUT)).astype(BF16)
    wdw = np.ascontiguousarray(wd.reshape(OT, 128, 9))

    # fp8 DoubleRow stationary: wq8[k, j, m] = wq[m, 128j + k], m < 768
    wq8 = np.zeros((128, 2, CQK), np.float32)
    for j in range(2):
        wq8[:, j, :] = wq[:CQK, 128 * j : 128 * (j + 1)].T
    wq8 = np.ascontiguousarray(wq8.reshape(128, 2 * CQK)).astype(FP8)


    # wprojH[i, h*384 + o] = proj_w[o, 48h + i]
    wprojH = np.zeros((HD, HEADS * DIM), np.float32)
    for h in range(HEADS):
        wprojH[:, h * DIM : (h + 1) * DIM] = proj_w[:, h * HD : (h + 1) * HD].T
    wprojH = wprojH.astype(BF16)

    tempb = np.broadcast_to(temp[None, :], (HD, HEADS)).copy()
    eyeb = np.eye(128, dtype=np.float32).astype(BF16)
    eyem = np.tile(np.eye(96, dtype=np.float32), (1, 8)).copy()
    mask8 = np.repeat(np.eye(HEADS, dtype=np.float32), HD, axis=1).astype(BF16)
    wdiag = np.zeros((OT, len(WDIAG_TAPS), 128, 128), np.float32)
    for ot in range(OT):
        for i, s in enumerate(WDIAG_TAPS):
            np.fill_diagonal(wdiag[ot, i], wd[ot * 128 : (ot + 1) * 128, s])
    # sbuf layout: [128 part(k), ntaps*128 free(s, m)]
    wdiag = np.ascontiguousarray(wdiag.transpose(0, 2, 1, 3)).reshape(
        OT, 128, len(WDIAG_TAPS) * 128
    ).astype(BF16)
    return {
        "wqkvT": wqkvT,
        "wq8": wq8,
        "wdw": wdw,
        "wprojH": wprojH,
        "tempb": tempb,
        "eyeb": eyeb,
        "eyem": eyem,
        "mask8": mask8,
        "wdiag": wdiag,
    }


def _prep_x(x):
    """Per-core padded x chunks: bf16 [CT, 128, HP*WP] and the fp8
    interleaved copy of c-tiles 0,1 ([128, 2*X8W])."""
    x = np.asarray(x, np.float32)
    chunks = []
    for c in range(N_CORES):
        b, r0 = c // 4, ROWS * (c % 4)
        buf = np.zeros((DIM, HP, WP), np.float32)
        lo, hi = max(r0 - 1, 0), min(r0 + ROWS + 1, HGT)
        buf[:, lo - (r0 - 1) : hi - (r0 - 1), 1 : WID + 1] = x[b, :, lo:hi, :]
        flat = buf.reshape(CT, 128, HP * WP)
        x8 = np.zeros((128, 2, X8W), np.float32)
        x8[:, :, : HP * WP] = flat[:2].transpose(1, 0, 2)
        chunks.append((
            np.ascontiguousarray(flat).astype(BF16),
            np.ascontiguousarray(x8.reshape(128, 2 * X8W)).astype(FP8),
        ))
    return chunks


def _run(x, qkv_w, dw_w, proj_w, log_temp, trace=False):
    nc = _get_program()
    shared = _prep_weights(qkv_w, dw_w, proj_w, log_temp)
    xchunks = _prep_x(x)
    in_maps = [
        {**shared, "xp": xchunks[c][0], "xp8": xchunks[c][1]}
        for c in range(N_CORES)
    ]
    res = run_bass_kernel_spmd(
        nc, in_maps, core_ids=list(range(N_CORES)), trace=trace
    )
    out = np.empty((B, DIM, HGT, WID), np.float32)
    for c in range(N_CORES):
        b, r0 = c // 4, ROWS * (c % 4)
        yc = res.results[c]["y"].reshape(DIM, ROWS, WID)
        out[b, :, r0 : r0 + ROWS, :] = yc
    return out, res


def kernel(x, qkv_w, dw_w, proj_w, log_temp):
    out, _ = _run(x, qkv_w, dw_w, proj_w, log_temp, trace=False)
    return out
